# revision 10
# baseline (speedup 1.0000x reference)
"""Trainium2 Bass kernel for nn_Attention_43413529428606 (linear attention
with l2-normed q/k, interleaved RoPE, mask, per-head power scaling).

Sharding: the 16384 (batch*seq) rows are split across 8 NeuronCores, 2048
rows each; cores 0-3 take batch 0, cores 4-7 batch 1.  Each core computes
q/k/v projections for its rows (all 16 heads), applies l2norm+RoPE+mask,
accumulates the per-head k^T v state, AllReduces that state (512 KB) within
its batch group, then applies attention and the output projection for its
rows.  Everything is fused on-chip; only q-hat spills to DRAM.

Self-contained: hardcodes all shapes; no sibling imports.
"""

import sys

for _p in ("/opt/trn_rl_repo",):
    if _p not in sys.path:
        sys.path.append(_p)

from contextlib import ExitStack

import numpy as np

import concourse.bass as bass
import concourse.bacc as bacc
import concourse.tile as tile
from concourse import mybir
from concourse.bass_utils import run_bass_kernel_spmd

F32 = mybir.dt.float32
F32R = mybir.dt.float32r
BF16 = mybir.dt.bfloat16

DIM = 1024
H = 16
HD = 64
B = 2
C = 8192
ROPE_THETA = 10000.0

N_CORES = 8
R = (B * C) // N_CORES  # 2048 rows per core
NC_T = R // 128  # 16 c-tiles of 128 (phase A)
NQ_T = R // 512  # 4 c-supertiles of 512 (phases B/C)
ND = DIM // 128  # 8 d-chunks
NJ = DIM // 128  # 8 j-tiles
NPAIR = H // 2  # 8 head pairs

Copy = mybir.ActivationFunctionType.Copy
Square = mybir.ActivationFunctionType.Square
Ln = mybir.ActivationFunctionType.Ln
Exp = mybir.ActivationFunctionType.Exp
MUL = mybir.AluOpType.mult
ADD = mybir.AluOpType.add


def build_nc():
    nc = bacc.Bacc(
        "TRN2", target_bir_lowering=False, debug=False, num_devices=N_CORES
    )

    # ---- DRAM parameters (per-core shapes) ----
    xT = nc.dram_tensor("xT", [DIM, R], F32R, kind="ExternalInput").ap()
    WkT = nc.dram_tensor("WkT", [DIM, DIM], F32R, kind="ExternalInput").ap()
    WvT = nc.dram_tensor("WvT", [DIM, DIM], F32R, kind="ExternalInput").ap()
    WqT = nc.dram_tensor("WqT", [DIM, DIM], F32R, kind="ExternalInput").ap()
    WoT = nc.dram_tensor("WoT", [DIM, DIM], F32R, kind="ExternalInput").ap()
    cosC = nc.dram_tensor("cosC", [R, HD], F32, kind="ExternalInput").ap()
    sinC = nc.dram_tensor("sinC", [R, HD], F32, kind="ExternalInput").ap()
    cosF = nc.dram_tensor("cosF", [128, R], F32, kind="ExternalInput").ap()
    sinF = nc.dram_tensor("sinF", [128, R], F32, kind="ExternalInput").ap()
    maskF = nc.dram_tensor("maskF", [128, R], F32, kind="ExternalInput").ap()
    maskC = nc.dram_tensor("maskC", [128, NC_T], F32, kind="ExternalInput").ap()
    ind16T = nc.dram_tensor("ind16T", [DIM, 16], F32R, kind="ExternalInput").ap()
    ind16 = nc.dram_tensor("ind16", [16, DIM], F32R, kind="ExternalInput").ap()
    Pmat = nc.dram_tensor("Pmat", [128, 128], F32R, kind="ExternalInput").ap()
    zeros64 = nc.dram_tensor("zeros64", [64, 64], F32R, kind="ExternalInput").ap()

    qhat_d = nc.dram_tensor("qhat_d", [DIM, R], F32R)
    kv_in_d = nc.dram_tensor("kv_in_d", [128, NPAIR * 128], F32)
    kv_out_d = nc.dram_tensor("kv_out_d", [128, NPAIR * 128], F32)

    out_d = nc.dram_tensor("out", [DIM, R], F32, kind="ExternalOutput").ap()
    dbg_khat = nc.dram_tensor("dbg_khat", [128, DIM], F32, kind="ExternalOutput").ap()
    dbg_v = nc.dram_tensor("dbg_v", [128, DIM], F32, kind="ExternalOutput").ap()
    dbg_kv = nc.dram_tensor("dbg_kv", [128, DIM], F32, kind="ExternalOutput").ap()
    dbg_kvout = nc.dram_tensor("dbg_kvout", [128, DIM], F32, kind="ExternalOutput").ap()
    dbg_qhat = nc.dram_tensor("dbg_qhat", [128, 512], F32, kind="ExternalOutput").ap()
    dbg_attn = nc.dram_tensor("dbg_attn", [128, 512], F32, kind="ExternalOutput").ap()

    with tile.TileContext(nc) as tc:
        with ExitStack() as ctx:
            consts = ctx.enter_context(tc.tile_pool(name="consts", bufs=1))
            xpool = ctx.enter_context(tc.tile_pool(name="xpool", bufs=1))
            kvblk_pool = ctx.enter_context(tc.tile_pool(name="kvblk", bufs=1))

            # ---- load x and small constants ----
            xT_t = []
            for dc in range(ND):
                t = xpool.tile([128, R], F32R, tag=f"xT{dc}")
                nc.sync.dma_start(out=t[:], in_=xT[dc * 128 : (dc + 1) * 128, :])
                xT_t.append(t)

            cosC_t = consts.tile([128, NC_T * HD], F32, tag="cosC")
            sinC_t = consts.tile([128, NC_T * HD], F32, tag="sinC")
            # [R, 64] row-major -> tiles of [128 rows, 64] side by side
            nc.sync.dma_start(
                out=cosC_t[:].rearrange("p (t f) -> p t f", t=NC_T),
                in_=cosC[:].rearrange("(t p) f -> p t f", p=128),
            )
            nc.sync.dma_start(
                out=sinC_t[:].rearrange("p (t f) -> p t f", t=NC_T),
                in_=sinC[:].rearrange("(t p) f -> p t f", p=128),
            )
            maskC_t = consts.tile([128, NC_T], F32, tag="maskC")
            ind16T_t = consts.tile([128, NJ * 16], F32R, tag="ind16T")
            ind16_t = consts.tile([16, DIM], F32R, tag="ind16")
            P_t = consts.tile([128, 128], F32R, tag="Pmat")
            nc.sync.dma_start(out=maskC_t[:], in_=maskC[:])
            nc.sync.dma_start(
                out=ind16T_t[:].rearrange("p (t f) -> p t f", t=NJ),
                in_=ind16T[:].rearrange("(t p) f -> p t f", p=128),
            )
            nc.sync.dma_start(out=ind16_t[:], in_=ind16[:])
            nc.sync.dma_start(out=P_t[:], in_=Pmat[:])

            # =============== Phase A: k/v proj + process + kv Grams ========
            with ExitStack() as ctxA:
                wA = ctxA.enter_context(tc.tile_pool(name="wA", bufs=1))
                psA = ctxA.enter_context(
                    tc.tile_pool(name="psA", bufs=3, space="PSUM")
                )
                pskv = ctxA.enter_context(
                    tc.tile_pool(name="pskv", bufs=1, space="PSUM")
                )
                sbA = ctxA.enter_context(tc.tile_pool(name="sbA", bufs=2))
                smA = ctxA.enter_context(tc.tile_pool(name="smA", bufs=2))

                WkT_t, WvT_t = [], []
                for dc in range(ND):
                    tk = wA.tile([128, DIM], F32R, tag=f"WkT{dc}")
                    tv = wA.tile([128, DIM], F32R, tag=f"WvT{dc}")
                    nc.sync.dma_start(
                        out=tk[:], in_=WkT[dc * 128 : (dc + 1) * 128, :]
                    )
                    nc.sync.dma_start(
                        out=tv[:], in_=WvT[dc * 128 : (dc + 1) * 128, :]
                    )
                    WkT_t.append(tk)
                    WvT_t.append(tv)

                kv_ps = pskv.tile([128, NPAIR * 128], F32, tag="kvps")

                for ct in range(NC_T):
                    cs = slice(ct * 128, (ct + 1) * 128)
                    k_ps = psA.tile([128, DIM], F32, tag="proj_ps")
                    v_ps = psA.tile([128, DIM], F32, tag="proj_ps")
                    for half in range(2):
                        js = slice(half * 512, (half + 1) * 512)
                        for dc in range(ND):
                            nc.tensor.matmul(
                                k_ps[:, js],
                                xT_t[dc][:, cs],
                                WkT_t[dc][:, js],
                                start=(dc == 0),
                                stop=(dc == ND - 1),
                            )
                        for dc in range(ND):
                            nc.tensor.matmul(
                                v_ps[:, js],
                                xT_t[dc][:, cs],
                                WvT_t[dc][:, js],
                                start=(dc == 0),
                                stop=(dc == ND - 1),
                            )

                    # v: evict with mask fold (per-partition scale)
                    v_sb = sbA.tile([128, DIM], F32R, tag="v_sb")
                    nc.scalar.activation(
                        v_sb[:], v_ps[:], Copy, scale=maskC_t[:, ct : ct + 1]
                    )

                    # k chain (c-major): rope + l2norm + mask
                    cosb = (
                        cosC_t[:, ct * HD : (ct + 1) * HD]
                        .unsqueeze(1)
                        .broadcast_to([128, H, HD])
                    )
                    k3 = k_ps[:].rearrange("p (h f) -> p h f", h=H)
                    k_sw = k_ps[:].rearrange(
                        "p (h g two) -> p h g two", h=H, two=2
                    )[:, :, :, ::-1]
                    sinb4 = (
                        sinC_t[:, ct * HD : (ct + 1) * HD]
                        .rearrange("p (g two) -> p g two", two=2)
                        .unsqueeze(1)
                        .broadcast_to([128, H, HD // 2, 2])
                    )

                    m1 = sbA.tile([128, DIM], F32, tag="m1")
                    nc.vector.tensor_tensor(
                        m1[:].rearrange("p (h f) -> p h f", h=H), k3, cosb, MUL
                    )
                    sq = sbA.tile([128, DIM], F32, tag="sq")
                    nc.scalar.activation(sq[:], k_ps[:], Square)
                    red = smA.tile([128, H], F32, tag="red")
                    nc.vector.tensor_reduce(
                        red[:],
                        sq[:].rearrange("p (h f) -> p h f", h=H),
                        mybir.AxisListType.X,
                        ADD,
                    )
                    lnr = smA.tile([128, H], F32, tag="lnr")
                    nc.scalar.activation(lnr[:], red[:], Ln)
                    rs = smA.tile([128, H], F32, tag="rs")
                    nc.scalar.activation(rs[:], lnr[:], Exp, scale=-0.5)
                    rsm = smA.tile([128, H], F32, tag="rsm")
                    nc.vector.tensor_scalar_mul(
                        rsm[:], rs[:], maskC_t[:, ct : ct + 1]
                    )
                    m2 = sbA.tile([128, DIM], F32, tag="m2")
                    nc.vector.tensor_tensor(
                        m2[:].rearrange("p (h g two) -> p h g two", h=H, two=2),
                        k_sw,
                        sinb4,
                        MUL,
                    )
                    s = sbA.tile([128, DIM], F32, tag="s")
                    nc.vector.tensor_tensor(s[:], m1[:], m2[:], ADD)
                    khat = sbA.tile([128, DIM], F32R, tag="khat")
                    rsb = rsm[:].unsqueeze(2).broadcast_to([128, H, HD])
                    nc.vector.tensor_tensor(
                        khat[:].rearrange("p (h f) -> p h f", h=H),
                        s[:].rearrange("p (h f) -> p h f", h=H),
                        rsb,
                        MUL,
                    )

                    if ct == 0:
                        nc.sync.dma_start(out=dbg_khat[:], in_=khat[:].bitcast(F32))
                        nc.sync.dma_start(out=dbg_v[:], in_=v_sb[:].bitcast(F32))

                    # kv pair-Gram accumulation
                    for p in range(NPAIR):
                        ps_ = slice(p * 128, (p + 1) * 128)
                        # start=True zeroes the whole PSUM bank, so only the
                        # first pair written to each bank may carry it
                        nc.tensor.matmul(
                            kv_ps[:, ps_],
                            khat[:, ps_],
                            v_sb[:, ps_],
                            start=(ct == 0 and p % 4 == 0),
                            stop=(ct == NC_T - 1),
                        )

                # evict kv partials and run the collective
                kv_sb = sbA.tile([128, NPAIR * 128], F32, tag="kv_sb")
                nc.vector.tensor_copy(kv_sb[:], kv_ps[:])
                nc.sync.dma_start(out=kv_in_d.ap(), in_=kv_sb[:])
                nc.sync.dma_start(out=dbg_kv[:], in_=kv_sb[:])
                nc.gpsimd.collective_compute(
                    "AllReduce",
                    ADD,
                    replica_groups=[[0, 1, 2, 3], [4, 5, 6, 7]],
                    ins=[kv_in_d.ap().opt()],
                    outs=[kv_out_d.ap().opt()],
                )

            # kvblk: zeroed [128,128] per pair, diag blocks DMAed in
            kvb_sb_dbg = kvblk_pool.tile([128, DIM], F32, tag="kvdbg")
            nc.sync.dma_start(out=kvb_sb_dbg[:], in_=kv_out_d.ap())
            nc.sync.dma_start(out=dbg_kvout[:], in_=kvb_sb_dbg[:])
            kvblk = []
            for p in range(NPAIR):
                t = kvblk_pool.tile([128, 128], F32R, tag=f"kvblk{p}")
                nc.sync.dma_start(out=t[0:64, 64:128], in_=zeros64[:])
                nc.sync.dma_start(out=t[64:128, 0:64], in_=zeros64[:])
                nc.sync.dma_start(
                    out=t[0:64, 0:64],
                    in_=kv_out_d.ap()[0:64, p * 128 : p * 128 + 64].bitcast(F32R),
                )
                nc.sync.dma_start(
                    out=t[64:128, 64:128],
                    in_=kv_out_d.ap()[
                        64:128, p * 128 + 64 : p * 128 + 128
                    ].bitcast(F32R),
                )
                kvblk.append(t)

            # =============== Phase B: q proj + l2norm + rope -> qhat_d =====
            with ExitStack() as ctxB:
                wB = ctxB.enter_context(tc.tile_pool(name="wB", bufs=1))
                psB = ctxB.enter_context(
                    tc.tile_pool(name="psB", bufs=2, space="PSUM")
                )
                psN = ctxB.enter_context(
                    tc.tile_pool(name="psN", bufs=2, space="PSUM")
                )
                sbB = ctxB.enter_context(tc.tile_pool(name="sbB", bufs=3))
                sbS = ctxB.enter_context(tc.tile_pool(name="sbS", bufs=2 * NJ))

                WqT_t = []
                for dc in range(ND):
                    t = wB.tile([128, DIM], F32R, tag=f"WqT{dc}")
                    nc.sync.dma_start(
                        out=t[:], in_=WqT[dc * 128 : (dc + 1) * 128, :]
                    )
                    WqT_t.append(t)
                cosF_t = wB.tile([128, R], F32, tag="cosF")
                sinF_t = wB.tile([128, R], F32, tag="sinF")
                nc.sync.dma_start(out=cosF_t[:], in_=cosF[:])
                nc.sync.dma_start(out=sinF_t[:], in_=sinF[:])

                for ct in range(NQ_T):
                    cs = slice(ct * 512, (ct + 1) * 512)
                    norms_ps = psN.tile([16, 512], F32, tag="norms")
                    s_tiles = []
                    for jt in range(NJ):
                        jsl = slice(jt * 128, (jt + 1) * 128)
                        q_ps = psB.tile([128, 512], F32, tag="q_ps")
                        for dc in range(ND):
                            nc.tensor.matmul(
                                q_ps[:],
                                WqT_t[dc][:, jsl],
                                xT_t[dc][:, cs],
                                start=(dc == 0),
                                stop=(dc == ND - 1),
                            )
                        q_sb = sbB.tile([128, 512], F32R, tag="q_sb")
                        nc.scalar.activation(q_sb[:], q_ps[:], Copy)
                        sq = sbB.tile([128, 512], F32R, tag="sqB")
                        nc.scalar.activation(sq[:], q_sb[:], Square)
                        nc.tensor.matmul(
                            norms_ps[:],
                            ind16T_t[:, jt * 16 : (jt + 1) * 16],
                            sq[:],
                            start=(jt == 0),
                            stop=(jt == NJ - 1),
                        )
                        rot_ps = psB.tile([128, 512], F32, tag="rot_ps")
                        nc.tensor.matmul(
                            rot_ps[:], P_t[:], q_sb[:], start=True, stop=True
                        )
                        t1 = sbB.tile([128, 512], F32, tag="t1")
                        nc.vector.tensor_tensor(
                            t1[:], q_sb[:].bitcast(F32), cosF_t[:, cs], MUL
                        )
                        t2 = sbB.tile([128, 512], F32, tag="t2")
                        nc.vector.tensor_tensor(
                            t2[:], rot_ps[:], sinF_t[:, cs], MUL
                        )
                        s = sbS.tile([128, 512], F32, tag="sB")
                        nc.vector.tensor_tensor(s[:], t1[:], t2[:], ADD)
                        s_tiles.append(s)

                    lnn = sbB.tile([16, 512], F32, tag="lnn")
                    nc.scalar.activation(lnn[:], norms_ps[:], Ln)
                    rs16 = sbB.tile([16, 512], F32R, tag="rs16")
                    nc.scalar.activation(rs16[:], lnn[:], Exp, scale=-0.5)

                    for jt in range(NJ):
                        jsl = slice(jt * 128, (jt + 1) * 128)
                        rep_ps = psB.tile([128, 512], F32, tag="rep_ps")
                        nc.tensor.matmul(
                            rep_ps[:],
                            ind16_t[:, jsl],
                            rs16[:],
                            start=True,
                            stop=True,
                        )
                        qhat = sbB.tile([128, 512], F32R, tag="qhat")
                        nc.vector.tensor_tensor(
                            qhat[:], s_tiles[jt][:], rep_ps[:], MUL
                        )
                        nc.sync.dma_start(out=qhat_d.ap()[jsl, cs], in_=qhat[:])
                        if ct == 0 and jt == 0:
                            nc.sync.dma_start(out=dbg_qhat[:], in_=qhat[:].bitcast(F32))

            # =============== Phase C: attn + out proj ======================
            with ExitStack() as ctxC:
                wC = ctxC.enter_context(tc.tile_pool(name="wC", bufs=1))
                psC = ctxC.enter_context(
                    tc.tile_pool(name="psC", bufs=2, space="PSUM")
                )
                psO = ctxC.enter_context(
                    tc.tile_pool(name="psO", bufs=2, space="PSUM")
                )
                sbC = ctxC.enter_context(tc.tile_pool(name="sbC", bufs=3))
                sbAt = ctxC.enter_context(tc.tile_pool(name="sbAt", bufs=2 * NJ))

                WoT_t = []
                for dc in range(ND):
                    t = wC.tile([128, DIM], F32R, tag=f"WoT{dc}")
                    nc.sync.dma_start(
                        out=t[:], in_=WoT[dc * 128 : (dc + 1) * 128, :]
                    )
                    WoT_t.append(t)
                maskF_t = wC.tile([128, R], F32, tag="maskF")
                nc.sync.dma_start(out=maskF_t[:], in_=maskF[:])

                for ct in range(NQ_T):
                    cs = slice(ct * 512, (ct + 1) * 512)
                    attn_sb = []
                    for hp in range(NPAIR):
                        jsl = slice(hp * 128, (hp + 1) * 128)
                        qh = sbC.tile([128, 512], F32R, tag="qh")
                        nc.sync.dma_start(out=qh[:], in_=qhat_d.ap()[jsl, cs])
                        a_ps = psC.tile([128, 512], F32, tag="a_ps")
                        nc.tensor.matmul(
                            a_ps[:], kvblk[hp][:], qh[:], start=True, stop=True
                        )
                        a_sb = sbAt.tile([128, 512], F32R, tag="a_sb")
                        nc.scalar.activation(a_sb[:], a_ps[:], Copy)
                        if ct == 0 and hp == 0:
                            nc.sync.dma_start(out=dbg_attn[:], in_=a_sb[:].bitcast(F32))
                        attn_sb.append(a_sb)

                    for et in range(NJ):
                        esl = slice(et * 128, (et + 1) * 128)
                        o_ps = psO.tile([128, 512], F32, tag="o_ps")
                        for jt in range(NJ):
                            nc.tensor.matmul(
                                o_ps[:],
                                WoT_t[jt][:, esl],
                                attn_sb[jt][:],
                                start=(jt == 0),
                                stop=(jt == NJ - 1),
                            )
                        o_sb = sbC.tile([128, 512], F32, tag="o_sb")
                        nc.vector.tensor_tensor(
                            o_sb[:], o_ps[:], maskF_t[:, cs], MUL
                        )
                        nc.sync.dma_start(out=out_d[esl, cs], in_=o_sb[:])

    nc.compile()
    return nc


_NC_CACHE = None


def _get_nc():
    global _NC_CACHE
    if _NC_CACHE is None:
        _NC_CACHE = build_nc()
    return _NC_CACHE


def make_in_maps(x, mask, Wq, Wk, Wv, Wo, norm_const):
    x = np.asarray(x, np.float32)
    mask = np.asarray(mask)
    Wq = np.asarray(Wq, np.float32)
    Wk = np.asarray(Wk, np.float32)
    Wv = np.asarray(Wv, np.float32)
    Wo = np.asarray(Wo, np.float32)
    norm_const = np.asarray(norm_const, np.float32).reshape(H)

    sig = 1.0 / (1.0 + np.exp(-norm_const.astype(np.float64)))
    svec = np.float64(C) ** (-sig)  # [H]
    s_cols = np.repeat(svec, HD)  # [DIM]

    WkT = np.ascontiguousarray(Wk.T)
    WvT = np.ascontiguousarray((Wv * s_cols[:, None].astype(np.float32)).T)
    WqT = np.ascontiguousarray(Wq.T)
    WoT = np.ascontiguousarray(Wo.T)

    # rope tables
    inv_freq = 1.0 / (
        ROPE_THETA ** (np.arange(0, HD, 2, dtype=np.float64) / HD)
    )  # [32]
    freq_of_j = np.repeat(inv_freq, 2)  # [64] interleaved

    ind16T = np.zeros((DIM, 16), np.float32)
    for jt in range(NJ):
        for kk in range(128):
            ind16T[jt * 128 + kk, 2 * jt + (kk >= 64)] = 1.0

    ind16 = np.zeros((16, DIM), np.float32)
    for jt in range(NJ):
        for m in range(128):
            ind16[2 * jt + (m >= 64), jt * 128 + m] = 1.0

    zeros64 = np.zeros((64, 64), np.float32)

    Pmat = np.zeros((128, 128), np.float32)
    for i in range(64):
        Pmat[2 * i + 1, 2 * i] = -1.0  # out[2i] = -q[2i+1]
        Pmat[2 * i, 2 * i + 1] = 1.0  # out[2i+1] = q[2i]

    in_maps = []
    for core in range(N_CORES):
        b = core // (N_CORES // B)
        cc = core % (N_CORES // B)
        c0 = cc * R
        pos = (c0 + np.arange(R)).astype(np.float64)

        xT = np.ascontiguousarray(x[b, c0 : c0 + R, :].T)

        angC = pos[:, None] * freq_of_j[None, :]  # [R, 64]
        cosC = np.cos(angC).astype(np.float32)
        sinC_ = np.sin(angC).astype(np.float32)
        # sign fold for the swap formulation: even j -> -sin, odd j -> +sin
        sinC_[:, 0::2] *= -1.0

        angF = freq_of_j[:, None] * pos[None, :]  # [64, R]
        angF2 = np.concatenate([angF, angF], axis=0)  # [128, R]
        cosF = np.cos(angF2).astype(np.float32)
        sinF = np.sin(angF2).astype(np.float32)

        mrow = mask[b, c0 : c0 + R].astype(np.float32)  # [R]
        maskF = np.broadcast_to(mrow[None, :], (128, R)).copy()
        maskC = np.ascontiguousarray(mrow.reshape(NC_T, 128).T)  # [128, NC_T]

        in_maps.append(
            {
                "xT": xT,
                "WkT": WkT,
                "WvT": WvT,
                "WqT": WqT,
                "WoT": WoT,
                "cosC": cosC,
                "sinC": sinC_,
                "cosF": cosF,
                "sinF": sinF,
                "maskF": maskF,
                "maskC": maskC,
                "ind16T": ind16T,
                "ind16": ind16,
                "Pmat": Pmat,
                "zeros64": zeros64,
            }
        )
    return in_maps


def assemble_output(results):
    out = np.empty((B, C, DIM), np.float32)
    for core in range(N_CORES):
        b = core // (N_CORES // B)
        cc = core % (N_CORES // B)
        c0 = cc * R
        out[b, c0 : c0 + R, :] = results[core]["out"].T
    return out


def kernel(x, mask, Wq, Wk, Wv, Wo, norm_const):
    nc = _get_nc()
    in_maps = make_in_maps(x, mask, Wq, Wk, Wv, Wo, norm_const)
    res = run_bass_kernel_spmd(nc, in_maps, list(range(N_CORES)))
    return assemble_output(res.results)


# revision 25
# speedup vs baseline: 1.6459x; 1.6459x over previous
"""Trainium2 Bass kernel for nn_Attention_43413529428606 (linear attention
with l2-normed q/k, interleaved RoPE, mask, per-head power scaling).

Sharding: the 16384 (batch*seq) rows are split across 8 NeuronCores, 2048
rows each; cores 0-3 take batch 0, cores 4-7 batch 1.  Each core computes
q/k/v projections for its rows (all 16 heads), applies l2norm+RoPE+mask,
accumulates the per-head k^T v state, AllReduces that state (512 KB) within
its batch group, then applies attention and the output projection for its
rows.  The data path is fp16 (fp32 PSUM accumulation); q/attn/out phases
are fused per 512-row supertile so nothing spills to DRAM.  The q-side
mask is applied host-side on the output rows.

Self-contained: hardcodes all shapes; no sibling imports.
"""

import sys

for _p in ("/opt/trn_rl_repo",):
    if _p not in sys.path:
        sys.path.append(_p)

from contextlib import ExitStack

import numpy as np

import concourse.bass as bass
import concourse.bacc as bacc
import concourse.tile as tile
from concourse import mybir
from concourse.bass_utils import run_bass_kernel_spmd

F32 = mybir.dt.float32
F16 = mybir.dt.float16

DIM = 1024
H = 16
HD = 64
B = 2
C = 8192
ROPE_THETA = 10000.0

N_CORES = 8
R = (B * C) // N_CORES  # 2048 rows per core
NC_T = R // 128  # 16 c-tiles of 128 (phase A)
NQ_T = R // 512  # 4 c-supertiles of 512 (fused q/attn/out phase)
ND = DIM // 128  # 8 d-chunks
NJ = DIM // 128  # 8 j-tiles
NPAIR = H // 2  # 8 head pairs

Copy = mybir.ActivationFunctionType.Copy
Square = mybir.ActivationFunctionType.Square
Ln = mybir.ActivationFunctionType.Ln
Exp = mybir.ActivationFunctionType.Exp
MUL = mybir.AluOpType.mult
ADD = mybir.AluOpType.add


def build_nc(sim_mode=False, phases="ABC"):
    nc = bacc.Bacc(
        "TRN2",
        target_bir_lowering=False,
        debug=False,
        num_devices=1 if sim_mode else N_CORES,
    )

    # ---- DRAM parameters (per-core shapes, fp16 data path) ----
    xT = nc.dram_tensor("xT", [DIM, R], F16, kind="ExternalInput").ap()
    WkT = nc.dram_tensor("WkT", [DIM, DIM], F16, kind="ExternalInput").ap()
    WvT = nc.dram_tensor("WvT", [DIM, DIM], F16, kind="ExternalInput").ap()
    WqT = nc.dram_tensor("WqT", [DIM, DIM], F16, kind="ExternalInput").ap()
    WoT = nc.dram_tensor("WoT", [DIM, DIM], F16, kind="ExternalInput").ap()
    cosC = nc.dram_tensor("cosC", [R, HD], F16, kind="ExternalInput").ap()
    sinC = nc.dram_tensor("sinC", [R, HD], F16, kind="ExternalInput").ap()
    cosF = nc.dram_tensor("cosF", [128, R], F16, kind="ExternalInput").ap()
    sinF = nc.dram_tensor("sinF", [128, R], F16, kind="ExternalInput").ap()
    maskC = nc.dram_tensor("maskC", [128, NC_T], F32, kind="ExternalInput").ap()
    ind16T = nc.dram_tensor("ind16T", [DIM, 16], F16, kind="ExternalInput").ap()
    ind16 = nc.dram_tensor("ind16", [16, DIM], F16, kind="ExternalInput").ap()
    Pmat = nc.dram_tensor("Pmat", [128, 128], F16, kind="ExternalInput").ap()

    kv_in_d = nc.dram_tensor("kv_in_d", [128, NPAIR * 128], F32)
    kv_out_d = nc.dram_tensor("kv_out_d", [128, NPAIR * 128], F32)

    out_d = nc.dram_tensor("out", [DIM, R], F32, kind="ExternalOutput").ap()

    def blkview(dram_ap, csl):
        return dram_ap.rearrange("(t p) c -> p t c", p=128)[:, :, csl]

    with tile.TileContext(nc) as tc:
        with ExitStack() as ctx:
            consts = ctx.enter_context(tc.tile_pool(name="consts", bufs=1))
            kvblk_pool = ctx.enter_context(tc.tile_pool(name="kvblk", bufs=1))

            cosC_t = consts.tile([128, NC_T * HD], F16, tag="cosC")
            sinC_t = consts.tile([128, NC_T * HD], F16, tag="sinC")
            nc.sync.dma_start(
                out=cosC_t[:].rearrange("p (t f) -> p t f", t=NC_T),
                in_=cosC[:].rearrange("(t p) f -> p t f", p=128),
            )
            nc.sync.dma_start(
                out=sinC_t[:].rearrange("p (t f) -> p t f", t=NC_T),
                in_=sinC[:].rearrange("(t p) f -> p t f", p=128),
            )
            maskC_t = consts.tile([128, NC_T], F32, tag="maskC")
            ind16T_t = consts.tile([128, NJ * 16], F16, tag="ind16T")
            ind16_t = consts.tile([16, DIM], F16, tag="ind16")
            P_t = consts.tile([128, 128], F16, tag="Pmat")
            nc.sync.dma_start(out=maskC_t[:], in_=maskC[:])
            nc.sync.dma_start(
                out=ind16T_t[:].rearrange("p (t f) -> p t f", t=NJ),
                in_=ind16T[:].rearrange("(t p) f -> p t f", p=128),
            )
            nc.sync.dma_start(out=ind16_t[:], in_=ind16[:])
            nc.sync.dma_start(out=P_t[:], in_=Pmat[:])

            with ExitStack() as ctxX:
                xpool = ctxX.enter_context(tc.tile_pool(name="xpool", bufs=1))
                xT_all = xpool.tile([128, ND * R], F16, tag="xT")
                for xc in range(4):
                    nc.sync.dma_start(
                        out=xT_all[:, xc * 2 * R : (xc + 1) * 2 * R].rearrange(
                            "p (t c) -> p t c", t=2
                        ),
                        in_=xT[xc * 256 : (xc + 1) * 256, :].rearrange(
                            "(t p) c -> p t c", p=128
                        ),
                    )

                def xsl(dc, csl):
                    lo = dc * R
                    return xT_all[:, lo + csl.start : lo + csl.stop]

                if "B" in phases and "C" in phases:
                    wBC = ctxX.enter_context(tc.tile_pool(name="wBC", bufs=1))
                    wq_all = wBC.tile([128, ND * DIM], F16, tag="wq")
                    nc.sync.dma_start(
                        out=wq_all[:].rearrange("p (t f) -> p t f", t=ND),
                        in_=WqT[:].rearrange("(t p) f -> p t f", p=128),
                    )
                    wo_all = wBC.tile([128, ND * DIM], F16, tag="wo")
                    nc.scalar.dma_start(
                        out=wo_all[:].rearrange("p (t f) -> p t f", t=ND),
                        in_=WoT[:].rearrange("(t p) f -> p t f", p=128),
                    )
                    cosF_t = wBC.tile([128, R], F16, tag="cosF")
                    sinF_t = wBC.tile([128, R], F16, tag="sinF")
                    nc.sync.dma_start(out=cosF_t[:], in_=cosF[:])
                    nc.scalar.dma_start(out=sinF_t[:], in_=sinF[:])

                # ========= Phase A: k/v proj + process + kv Grams ==========
                with ExitStack() as ctxA:
                  if "A" in phases:
                    wA = ctxA.enter_context(tc.tile_pool(name="wA", bufs=1))
                    psA = ctxA.enter_context(
                        tc.tile_pool(name="psA", bufs=3, space="PSUM")
                    )
                    pskv = ctxA.enter_context(
                        tc.tile_pool(name="pskv", bufs=1, space="PSUM")
                    )
                    sbA = ctxA.enter_context(tc.tile_pool(name="sbA", bufs=2))
                    sb1 = ctxA.enter_context(tc.tile_pool(name="sb1", bufs=2))
                    smA = ctxA.enter_context(tc.tile_pool(name="smA", bufs=2))

                    wk_all = wA.tile([128, ND * DIM], F16, tag="wk")
                    wv_all = wA.tile([128, ND * DIM], F16, tag="wv")
                    for wt, wsrc in ((wk_all, WkT), (wv_all, WvT)):
                        for xc in range(2):
                            nc.scalar.dma_start(
                                out=wt[
                                    :, xc * 4 * DIM : (xc + 1) * 4 * DIM
                                ].rearrange("p (t f) -> p t f", t=4),
                                in_=wsrc[xc * 512 : (xc + 1) * 512, :].rearrange(
                                    "(t p) f -> p t f", p=128
                                ),
                            )

                    kv_ps = pskv.tile([128, NPAIR * 128], F32, tag="kvps")
                    kv_pending = []

                    # On HW start=True zeroes the whole PSUM bank, so only
                    # the first pair written to each bank may carry it.
                    def _emit_kv(item):
                        ct_, khat_, v_ = item
                        for p in range(NPAIR):
                            ps_ = slice(p * 128, (p + 1) * 128)
                            nc.tensor.matmul(
                                kv_ps[:, ps_],
                                khat_[:, ps_],
                                v_[:, ps_],
                                start=(
                                    True
                                    if sim_mode
                                    else (ct_ == 0 and p % 4 == 0)
                                ),
                                stop=(
                                    True if sim_mode else (ct_ == NC_T - 1)
                                ),
                            )

                    for ct in range(NC_T):
                        cs = slice(ct * 128, (ct + 1) * 128)
                        k_ps = psA.tile([128, DIM], F32, tag="proj_ps")
                        v_ps = psA.tile([128, DIM], F32, tag="proj_ps")
                        for half in range(2):
                            js = slice(half * 512, (half + 1) * 512)
                            for dc in range(ND):
                                nc.tensor.matmul(
                                    k_ps[:, js],
                                    xsl(dc, cs),
                                    wk_all[
                                        :, dc * DIM + js.start : dc * DIM + js.stop
                                    ],
                                    start=(dc == 0),
                                    stop=(dc == ND - 1),
                                )
                            for dc in range(ND):
                                nc.tensor.matmul(
                                    v_ps[:, js],
                                    xsl(dc, cs),
                                    wv_all[
                                        :, dc * DIM + js.start : dc * DIM + js.stop
                                    ],
                                    start=(dc == 0),
                                    stop=(dc == ND - 1),
                                )

                        # v: evict with mask fold (per-partition scale), cast
                        v_sb = sbA.tile([128, DIM], F16, tag="v_sb")
                        nc.scalar.activation(
                            v_sb[:], v_ps[:], Copy, scale=maskC_t[:, ct : ct + 1]
                        )
                        # k: evict fast (cast fp16) to free the PSUM slot
                        k_sb = sbA.tile([128, DIM], F16, tag="k_sb")
                        nc.scalar.activation(k_sb[:], k_ps[:], Copy)
                        # squares for the l2 norm
                        sq = sbA.tile([128, DIM], F16, tag="sq")
                        nc.scalar.activation(sq[:], k_ps[:], Square)

                        cosb = (
                            cosC_t[:, ct * HD : (ct + 1) * HD]
                            .unsqueeze(1)
                            .broadcast_to([128, H, HD])
                        )
                        sinb4 = (
                            sinC_t[:, ct * HD : (ct + 1) * HD]
                            .rearrange("p (g two) -> p g two", two=2)
                            .unsqueeze(1)
                            .broadcast_to([128, H, HD // 2, 2])
                        )
                        k3 = k_sb[:].rearrange("p (h f) -> p h f", h=H)
                        k_sw = k_sb[:].rearrange(
                            "p (h g two) -> p h g two", h=H, two=2
                        )[:, :, :, ::-1]

                        m1 = sb1.tile([128, DIM], F16, tag="m1")
                        nc.vector.tensor_tensor(
                            m1[:].rearrange("p (h f) -> p h f", h=H), k3, cosb, MUL
                        )
                        red = smA.tile([128, H], F32, tag="red")
                        nc.vector.tensor_reduce(
                            red[:],
                            sq[:].rearrange("p (h f) -> p h f", h=H),
                            mybir.AxisListType.X,
                            ADD,
                        )
                        lnr = smA.tile([128, H], F32, tag="lnr")
                        nc.scalar.activation(lnr[:], red[:], Ln)
                        rs = smA.tile([128, H], F32, tag="rs")
                        nc.scalar.activation(rs[:], lnr[:], Exp, scale=-0.5)
                        rsm = smA.tile([128, H], F32, tag="rsm")
                        nc.vector.tensor_scalar_mul(
                            rsm[:], rs[:], maskC_t[:, ct : ct + 1]
                        )
                        m2 = sb1.tile([128, DIM], F16, tag="m2")
                        nc.vector.tensor_tensor(
                            m2[:].rearrange("p (h g two) -> p h g two", h=H, two=2),
                            k_sw,
                            sinb4,
                            MUL,
                        )
                        s = sb1.tile([128, DIM], F16, tag="s")
                        nc.vector.tensor_tensor(s[:], m1[:], m2[:], ADD)
                        khat = sbA.tile([128, DIM], F16, tag="khat")
                        rsb = rsm[:].unsqueeze(2).broadcast_to([128, H, HD])
                        nc.vector.tensor_tensor(
                            khat[:].rearrange("p (h f) -> p h f", h=H),
                            s[:].rearrange("p (h f) -> p h f", h=H),
                            rsb,
                            MUL,
                        )

                        # kv Grams are issued one iteration late (software
                        # pipelining) so PE never waits on the khat chain
                        kv_pending.append((ct, khat, v_sb))
                        if len(kv_pending) > 1:
                            _emit_kv(kv_pending.pop(0))

                    while kv_pending:
                        _emit_kv(kv_pending.pop(0))

                    # evict kv partials and run the collective
                    kv_sb = sbA.tile([128, NPAIR * 128], F32, tag="kv_sb")
                    nc.vector.tensor_copy(kv_sb[:], kv_ps[:])
                    nc.sync.dma_start(out=kv_in_d.ap(), in_=kv_sb[:])
                    if sim_mode:
                        # stand-in for the AllReduce so TimelineSim can run
                        nc.sync.dma_start(out=kv_out_d.ap(), in_=kv_in_d.ap())
                    else:
                        nc.gpsimd.collective_compute(
                            "AllReduce",
                            ADD,
                            replica_groups=[[0, 1, 2, 3], [4, 5, 6, 7]],
                            ins=[kv_in_d.ap().opt()],
                            outs=[kv_out_d.ap().opt()],
                        )

                # kvblk: load reduced Grams, cast to fp16 block-diag
                kvblk = kvblk_pool.tile([128, NPAIR * 128], F16, tag="kvblk")
                if "C" in phases:
                    kvf = kvblk_pool.tile([128, NPAIR * 128], F32, tag="kvf")
                    nc.scalar.dma_start(out=kvf[:], in_=kv_out_d.ap())
                    nc.vector.memset(kvblk[:], 0.0)
                    # top-left diag blocks of each pair, then bottom-right
                    nc.vector.tensor_copy(
                        kvblk[0:64, :].rearrange("p (t f) -> p t f", t=NPAIR)[
                            :, :, 0:64
                        ],
                        kvf[0:64, :].rearrange("p (t f) -> p t f", t=NPAIR)[
                            :, :, 0:64
                        ],
                    )
                    nc.vector.tensor_copy(
                        kvblk[64:128, :].rearrange("p (t f) -> p t f", t=NPAIR)[
                            :, :, 64:128
                        ],
                        kvf[64:128, :].rearrange("p (t f) -> p t f", t=NPAIR)[
                            :, :, 64:128
                        ],
                    )

                # ==== Fused phase B+C: q proj/norm/rope + attn + out proj ===
                with ExitStack() as ctxB:
                  if "B" in phases and "C" in phases:
                    psB = ctxB.enter_context(
                        tc.tile_pool(name="psB", bufs=2, space="PSUM")
                    )
                    psN = ctxB.enter_context(
                        tc.tile_pool(name="psN", bufs=1, space="PSUM")
                    )
                    psAt = ctxB.enter_context(
                        tc.tile_pool(name="psAt", bufs=1, space="PSUM")
                    )
                    psO = ctxB.enter_context(
                        tc.tile_pool(name="psO", bufs=2, space="PSUM")
                    )
                    sbB = ctxB.enter_context(tc.tile_pool(name="sbB", bufs=3))
                    sbS = ctxB.enter_context(
                        tc.tile_pool(name="sbS", bufs=NJ + 1)
                    )
                    sbQ = ctxB.enter_context(tc.tile_pool(name="sbQ", bufs=2))
                    sbAt = ctxB.enter_context(
                        tc.tile_pool(name="sbAt", bufs=NJ + 2)
                    )

                    def _emit_attn_out(item):
                        ct_, qh_ = item
                        cs_ = slice(ct_ * 512, (ct_ + 1) * 512)
                        attn_sb = []
                        for hp in range(NPAIR):
                            a_ps = psAt.tile([128, 512], F32, tag="a_ps")
                            nc.tensor.matmul(
                                a_ps[:],
                                kvblk[:, hp * 128 : (hp + 1) * 128],
                                qh_[:, hp * 512 : (hp + 1) * 512],
                                start=True,
                                stop=True,
                            )
                            a_sb = sbAt.tile([128, 512], F16, tag="a_sb")
                            if hp % 2 == 0:
                                nc.scalar.activation(a_sb[:], a_ps[:], Copy)
                            else:
                                nc.vector.tensor_copy(a_sb[:], a_ps[:])
                            attn_sb.append(a_sb)

                        o_all = sbQ.tile([128, NJ * 512], F32, tag="o_all")
                        for et in range(NJ):
                            elo = et * 128
                            o_ps = psO.tile([128, 512], F32, tag="o_ps")
                            for jt in range(NJ):
                                nc.tensor.matmul(
                                    o_ps[:],
                                    wo_all[
                                        :, jt * DIM + elo : jt * DIM + elo + 128
                                    ],
                                    attn_sb[jt][:],
                                    start=(jt == 0),
                                    stop=(jt == NJ - 1),
                                )
                            nc.scalar.activation(
                                o_all[:, et * 512 : (et + 1) * 512], o_ps[:], Copy
                            )
                        nc.scalar.dma_start(
                            out=blkview(out_d, cs_),
                            in_=o_all[:].rearrange("p (t c) -> p t c", t=NJ),
                        )

                    at_pending = []
                    for ct in range(NQ_T):
                        cs = slice(ct * 512, (ct + 1) * 512)
                        norms_ps = psN.tile([16, 512], F32, tag="norms")
                        qh_all = sbQ.tile([128, NJ * 512], F16, tag="qhall")
                        q_sbs = []
                        # pass 1: projections + squares + norm accumulation
                        for jt in range(NJ):
                            jlo = jt * 128
                            q_ps = psB.tile([128, 512], F32, tag="q_ps")
                            for dc in range(ND):
                                nc.tensor.matmul(
                                    q_ps[:],
                                    wq_all[
                                        :, dc * DIM + jlo : dc * DIM + jlo + 128
                                    ],
                                    xsl(dc, cs),
                                    start=(dc == 0),
                                    stop=(dc == ND - 1),
                                )
                            q_sb = sbS.tile([128, 512], F16, tag="q_sb")
                            nc.scalar.activation(q_sb[:], q_ps[:], Copy)
                            sq = sbB.tile([128, 512], F16, tag="sqB")
                            nc.vector.tensor_mul(sq[:], q_sb[:], q_sb[:])
                            nc.tensor.matmul(
                                norms_ps[:],
                                ind16T_t[:, jt * 16 : (jt + 1) * 16],
                                sq[:],
                                start=(jt == 0),
                                stop=(jt == NJ - 1),
                            )
                            q_sbs.append(q_sb)

                        lnn = sbB.tile([16, 512], F32, tag="lnn")
                        nc.scalar.activation(lnn[:], norms_ps[:], Ln)
                        rs16 = sbB.tile([16, 512], F16, tag="rs16")
                        nc.scalar.activation(rs16[:], lnn[:], Exp, scale=-0.5)

                        # pass 2: rotation + rope + scale into qh_all
                        for jt in range(NJ):
                            q_sb = q_sbs[jt]
                            rot_ps = psB.tile([128, 512], F32, tag="rotrep")
                            nc.tensor.matmul(
                                rot_ps[:], P_t[:], q_sb[:], start=True, stop=True
                            )
                            rep_ps = psB.tile([128, 512], F32, tag="rotrep")
                            nc.tensor.matmul(
                                rep_ps[:],
                                ind16_t[:, jt * 128 : (jt + 1) * 128],
                                rs16[:],
                                start=True,
                                stop=True,
                            )
                            t1 = sbB.tile([128, 512], F16, tag="t1")
                            nc.vector.tensor_tensor(
                                t1[:], q_sb[:], cosF_t[:, cs], MUL
                            )
                            t2 = sbB.tile([128, 512], F16, tag="t2")
                            nc.vector.tensor_tensor(
                                t2[:], rot_ps[:], sinF_t[:, cs], MUL
                            )
                            s = sbB.tile([128, 512], F16, tag="sB")
                            nc.vector.tensor_tensor(s[:], t1[:], t2[:], ADD)
                            nc.vector.tensor_tensor(
                                qh_all[:, jt * 512 : (jt + 1) * 512],
                                s[:],
                                rep_ps[:],
                                MUL,
                            )

                        at_pending.append((ct, qh_all))
                        if len(at_pending) > 1:
                            _emit_attn_out(at_pending.pop(0))

                    while at_pending:
                        _emit_attn_out(at_pending.pop(0))

    nc.compile()
    return nc


_NC_CACHE = None


def _get_nc():
    global _NC_CACHE
    if _NC_CACHE is None:
        _NC_CACHE = build_nc()
    return _NC_CACHE


def make_in_maps(x, mask, Wq, Wk, Wv, Wo, norm_const):
    x = np.asarray(x, np.float32)
    mask = np.asarray(mask)
    Wq = np.asarray(Wq, np.float32)
    Wk = np.asarray(Wk, np.float32)
    Wv = np.asarray(Wv, np.float32)
    Wo = np.asarray(Wo, np.float32)
    norm_const = np.asarray(norm_const, np.float32).reshape(H)

    sig = 1.0 / (1.0 + np.exp(-norm_const.astype(np.float64)))
    svec = np.float64(C) ** (-sig)  # [H]
    s_cols = np.repeat(svec, HD)  # [DIM]

    f16 = np.float16
    WkT = np.ascontiguousarray(Wk.T).astype(f16)
    WvT = np.ascontiguousarray((Wv * s_cols[:, None].astype(np.float32)).T).astype(
        f16
    )
    WqT = np.ascontiguousarray(Wq.T).astype(f16)
    WoT = np.ascontiguousarray(Wo.T).astype(f16)

    inv_freq = 1.0 / (
        ROPE_THETA ** (np.arange(0, HD, 2, dtype=np.float64) / HD)
    )  # [32]
    freq_of_j = np.repeat(inv_freq, 2)  # [64] interleaved

    ind16T = np.zeros((DIM, 16), f16)
    for jt in range(NJ):
        for kk in range(128):
            ind16T[jt * 128 + kk, 2 * jt + (kk >= 64)] = 1.0

    ind16 = np.zeros((16, DIM), f16)
    for jt in range(NJ):
        for m in range(128):
            ind16[2 * jt + (m >= 64), jt * 128 + m] = 1.0

    Pmat = np.zeros((128, 128), f16)
    for i in range(64):
        Pmat[2 * i + 1, 2 * i] = -1.0  # out[2i] = -q[2i+1]
        Pmat[2 * i, 2 * i + 1] = 1.0  # out[2i+1] = q[2i]

    in_maps = []
    for core in range(N_CORES):
        b = core // (N_CORES // B)
        cc = core % (N_CORES // B)
        c0 = cc * R
        pos = (c0 + np.arange(R)).astype(np.float64)

        xTc = np.ascontiguousarray(x[b, c0 : c0 + R, :].T).astype(f16)

        angC = pos[:, None] * freq_of_j[None, :]  # [R, 64]
        cosCc = np.cos(angC).astype(f16)
        sinCc = np.sin(angC).astype(np.float32)
        # sign fold for the swap formulation: even j -> -sin, odd j -> +sin
        sinCc[:, 0::2] *= -1.0
        sinCc = sinCc.astype(f16)

        angF = freq_of_j[:, None] * pos[None, :]  # [64, R]
        angF2 = np.concatenate([angF, angF], axis=0)  # [128, R]
        cosFc = np.cos(angF2).astype(f16)
        sinFc = np.sin(angF2).astype(f16)

        mrow = mask[b, c0 : c0 + R].astype(np.float32)  # [R]
        maskCc = np.ascontiguousarray(mrow.reshape(NC_T, 128).T)  # [128, NC_T]

        in_maps.append(
            {
                "xT": xTc,
                "WkT": WkT,
                "WvT": WvT,
                "WqT": WqT,
                "WoT": WoT,
                "cosC": cosCc,
                "sinC": sinCc,
                "cosF": cosFc,
                "sinF": sinFc,
                "maskC": maskCc,
                "ind16T": ind16T,
                "ind16": ind16,
                "Pmat": Pmat,
            }
        )
    return in_maps


def assemble_output(results, mask):
    out = np.empty((B, C, DIM), np.float32)
    for core in range(N_CORES):
        b = core // (N_CORES // B)
        cc = core % (N_CORES // B)
        c0 = cc * R
        out[b, c0 : c0 + R, :] = results[core]["out"].T
    # q-side mask: masked rows produce zero output
    out *= np.asarray(mask)[:, :, None].astype(np.float32)
    return out


def kernel(x, mask, Wq, Wk, Wv, Wo, norm_const):
    nc = _get_nc()
    in_maps = make_in_maps(x, mask, Wq, Wk, Wv, Wo, norm_const)
    res = run_bass_kernel_spmd(nc, in_maps, list(range(N_CORES)))
    return assemble_output(res.results, mask)


# revision 26
# speedup vs baseline: 9547.5605x; 5800.7083x over previous
"""Trainium2 Bass kernel for nn_Attention_43413529428606 (linear attention
with l2-normed q/k, interleaved RoPE, mask, per-head power scaling).

Sharding: the 16384 (batch*seq) rows are split across 8 NeuronCores, 2048
rows each; cores 0-3 take batch 0, cores 4-7 batch 1.  Each core computes
q/k/v projections for its rows (all 16 heads), applies l2norm+RoPE+mask,
accumulates the per-head k^T v state, AllReduces that state (512 KB) within
its batch group, then applies attention and the output projection for its
rows.  The data path is fp16 (fp32 PSUM accumulation); q/attn/out phases
are fused per 512-row supertile so nothing spills to DRAM.  The q-side
mask is applied host-side on the output rows.

Self-contained: hardcodes all shapes; no sibling imports.
"""

import sys

for _p in ("/opt/trn_rl_repo",):
    if _p not in sys.path:
        sys.path.append(_p)

from contextlib import ExitStack

import numpy as np

import concourse.bass as bass
import concourse.bacc as bacc
import concourse.tile as tile
from concourse import mybir
from concourse.bass_utils import run_bass_kernel_spmd

F32 = mybir.dt.float32
F16 = mybir.dt.float16

DIM = 1024
H = 16
HD = 64
B = 2
C = 8192
ROPE_THETA = 10000.0

N_CORES = 8
R = (B * C) // N_CORES  # 2048 rows per core
NC_T = R // 128  # 16 c-tiles of 128 (phase A)
NQ_T = R // 512  # 4 c-supertiles of 512 (fused q/attn/out phase)
ND = DIM // 128  # 8 d-chunks
NJ = DIM // 128  # 8 j-tiles
NPAIR = H // 2  # 8 head pairs

Copy = mybir.ActivationFunctionType.Copy
Square = mybir.ActivationFunctionType.Square
Ln = mybir.ActivationFunctionType.Ln
Exp = mybir.ActivationFunctionType.Exp
MUL = mybir.AluOpType.mult
ADD = mybir.AluOpType.add


def build_nc(sim_mode=False, phases="ABC", reps=1):
    nc = bacc.Bacc(
        "TRN2",
        target_bir_lowering=False,
        debug=False,
        num_devices=1 if sim_mode else N_CORES,
    )

    # ---- DRAM parameters (per-core shapes, fp16 data path) ----
    xT = nc.dram_tensor("xT", [DIM, R], F16, kind="ExternalInput").ap()
    WkT = nc.dram_tensor("WkT", [DIM, DIM], F16, kind="ExternalInput").ap()
    WvT = nc.dram_tensor("WvT", [DIM, DIM], F16, kind="ExternalInput").ap()
    WqT = nc.dram_tensor("WqT", [DIM, DIM], F16, kind="ExternalInput").ap()
    WoT = nc.dram_tensor("WoT", [DIM, DIM], F16, kind="ExternalInput").ap()
    cosC = nc.dram_tensor("cosC", [R, HD], F16, kind="ExternalInput").ap()
    sinC = nc.dram_tensor("sinC", [R, HD], F16, kind="ExternalInput").ap()
    cosF = nc.dram_tensor("cosF", [128, R], F16, kind="ExternalInput").ap()
    sinF = nc.dram_tensor("sinF", [128, R], F16, kind="ExternalInput").ap()
    maskC = nc.dram_tensor("maskC", [128, NC_T], F32, kind="ExternalInput").ap()
    ind16T = nc.dram_tensor("ind16T", [DIM, 16], F16, kind="ExternalInput").ap()
    ind16 = nc.dram_tensor("ind16", [16, DIM], F16, kind="ExternalInput").ap()
    Pmat = nc.dram_tensor("Pmat", [128, 128], F16, kind="ExternalInput").ap()

    kv_in_d = nc.dram_tensor("kv_in_d", [128, NPAIR * 128], F32)
    kv_out_d = nc.dram_tensor("kv_out_d", [128, NPAIR * 128], F32)

    out_d = nc.dram_tensor("out", [DIM, R], F32, kind="ExternalOutput").ap()

    def blkview(dram_ap, csl):
        return dram_ap.rearrange("(t p) c -> p t c", p=128)[:, :, csl]

    with tile.TileContext(nc) as tc:
        with ExitStack() as ctx:
            consts = ctx.enter_context(tc.tile_pool(name="consts", bufs=1))
            kvblk_pool = ctx.enter_context(tc.tile_pool(name="kvblk", bufs=1))

            cosC_t = consts.tile([128, NC_T * HD], F16, tag="cosC")
            sinC_t = consts.tile([128, NC_T * HD], F16, tag="sinC")
            nc.sync.dma_start(
                out=cosC_t[:].rearrange("p (t f) -> p t f", t=NC_T),
                in_=cosC[:].rearrange("(t p) f -> p t f", p=128),
            )
            nc.sync.dma_start(
                out=sinC_t[:].rearrange("p (t f) -> p t f", t=NC_T),
                in_=sinC[:].rearrange("(t p) f -> p t f", p=128),
            )
            maskC_t = consts.tile([128, NC_T], F32, tag="maskC")
            ind16T_t = consts.tile([128, NJ * 16], F16, tag="ind16T")
            ind16_t = consts.tile([16, DIM], F16, tag="ind16")
            P_t = consts.tile([128, 128], F16, tag="Pmat")
            nc.sync.dma_start(out=maskC_t[:], in_=maskC[:])
            nc.sync.dma_start(
                out=ind16T_t[:].rearrange("p (t f) -> p t f", t=NJ),
                in_=ind16T[:].rearrange("(t p) f -> p t f", p=128),
            )
            nc.sync.dma_start(out=ind16_t[:], in_=ind16[:])
            nc.sync.dma_start(out=P_t[:], in_=Pmat[:])

            for _rep in range(reps):
              with ExitStack() as ctxX:
                xpool = ctxX.enter_context(tc.tile_pool(name="xpool", bufs=1))
                xT_all = xpool.tile([128, ND * R], F16, tag="xT")
                for xc in range(4):
                    nc.sync.dma_start(
                        out=xT_all[:, xc * 2 * R : (xc + 1) * 2 * R].rearrange(
                            "p (t c) -> p t c", t=2
                        ),
                        in_=xT[xc * 256 : (xc + 1) * 256, :].rearrange(
                            "(t p) c -> p t c", p=128
                        ),
                    )

                def xsl(dc, csl):
                    lo = dc * R
                    return xT_all[:, lo + csl.start : lo + csl.stop]

                if "B" in phases and "C" in phases:
                    wBC = ctxX.enter_context(tc.tile_pool(name="wBC", bufs=1))
                    wq_all = wBC.tile([128, ND * DIM], F16, tag="wq")
                    nc.sync.dma_start(
                        out=wq_all[:].rearrange("p (t f) -> p t f", t=ND),
                        in_=WqT[:].rearrange("(t p) f -> p t f", p=128),
                    )
                    wo_all = wBC.tile([128, ND * DIM], F16, tag="wo")
                    nc.scalar.dma_start(
                        out=wo_all[:].rearrange("p (t f) -> p t f", t=ND),
                        in_=WoT[:].rearrange("(t p) f -> p t f", p=128),
                    )
                    cosF_t = wBC.tile([128, R], F16, tag="cosF")
                    sinF_t = wBC.tile([128, R], F16, tag="sinF")
                    nc.sync.dma_start(out=cosF_t[:], in_=cosF[:])
                    nc.scalar.dma_start(out=sinF_t[:], in_=sinF[:])

                # ========= Phase A: k/v proj + process + kv Grams ==========
                with ExitStack() as ctxA:
                  if "A" in phases:
                    wA = ctxA.enter_context(tc.tile_pool(name="wA", bufs=1))
                    psA = ctxA.enter_context(
                        tc.tile_pool(name="psA", bufs=3, space="PSUM")
                    )
                    pskv = ctxA.enter_context(
                        tc.tile_pool(name="pskv", bufs=1, space="PSUM")
                    )
                    sbA = ctxA.enter_context(tc.tile_pool(name="sbA", bufs=2))
                    sb1 = ctxA.enter_context(tc.tile_pool(name="sb1", bufs=2))
                    smA = ctxA.enter_context(tc.tile_pool(name="smA", bufs=2))

                    wk_all = wA.tile([128, ND * DIM], F16, tag="wk")
                    wv_all = wA.tile([128, ND * DIM], F16, tag="wv")
                    for wt, wsrc in ((wk_all, WkT), (wv_all, WvT)):
                        for xc in range(2):
                            nc.scalar.dma_start(
                                out=wt[
                                    :, xc * 4 * DIM : (xc + 1) * 4 * DIM
                                ].rearrange("p (t f) -> p t f", t=4),
                                in_=wsrc[xc * 512 : (xc + 1) * 512, :].rearrange(
                                    "(t p) f -> p t f", p=128
                                ),
                            )

                    kv_ps = pskv.tile([128, NPAIR * 128], F32, tag="kvps")
                    kv_pending = []

                    # On HW start=True zeroes the whole PSUM bank, so only
                    # the first pair written to each bank may carry it.
                    def _emit_kv(item):
                        ct_, khat_, v_ = item
                        for p in range(NPAIR):
                            ps_ = slice(p * 128, (p + 1) * 128)
                            nc.tensor.matmul(
                                kv_ps[:, ps_],
                                khat_[:, ps_],
                                v_[:, ps_],
                                start=(
                                    True
                                    if sim_mode
                                    else (ct_ == 0 and p % 4 == 0)
                                ),
                                stop=(
                                    True if sim_mode else (ct_ == NC_T - 1)
                                ),
                            )

                    for ct in range(NC_T):
                        cs = slice(ct * 128, (ct + 1) * 128)
                        k_ps = psA.tile([128, DIM], F32, tag="proj_ps")
                        v_ps = psA.tile([128, DIM], F32, tag="proj_ps")
                        for half in range(2):
                            js = slice(half * 512, (half + 1) * 512)
                            for dc in range(ND):
                                nc.tensor.matmul(
                                    k_ps[:, js],
                                    xsl(dc, cs),
                                    wk_all[
                                        :, dc * DIM + js.start : dc * DIM + js.stop
                                    ],
                                    start=(dc == 0),
                                    stop=(dc == ND - 1),
                                )
                            for dc in range(ND):
                                nc.tensor.matmul(
                                    v_ps[:, js],
                                    xsl(dc, cs),
                                    wv_all[
                                        :, dc * DIM + js.start : dc * DIM + js.stop
                                    ],
                                    start=(dc == 0),
                                    stop=(dc == ND - 1),
                                )

                        # v: evict with mask fold (per-partition scale), cast
                        v_sb = sbA.tile([128, DIM], F16, tag="v_sb")
                        nc.scalar.activation(
                            v_sb[:], v_ps[:], Copy, scale=maskC_t[:, ct : ct + 1]
                        )
                        # k: evict fast (cast fp16) to free the PSUM slot
                        k_sb = sbA.tile([128, DIM], F16, tag="k_sb")
                        nc.scalar.activation(k_sb[:], k_ps[:], Copy)
                        # squares for the l2 norm
                        sq = sbA.tile([128, DIM], F16, tag="sq")
                        nc.scalar.activation(sq[:], k_ps[:], Square)

                        cosb = (
                            cosC_t[:, ct * HD : (ct + 1) * HD]
                            .unsqueeze(1)
                            .broadcast_to([128, H, HD])
                        )
                        sinb4 = (
                            sinC_t[:, ct * HD : (ct + 1) * HD]
                            .rearrange("p (g two) -> p g two", two=2)
                            .unsqueeze(1)
                            .broadcast_to([128, H, HD // 2, 2])
                        )
                        k3 = k_sb[:].rearrange("p (h f) -> p h f", h=H)
                        k_sw = k_sb[:].rearrange(
                            "p (h g two) -> p h g two", h=H, two=2
                        )[:, :, :, ::-1]

                        m1 = sb1.tile([128, DIM], F16, tag="m1")
                        nc.vector.tensor_tensor(
                            m1[:].rearrange("p (h f) -> p h f", h=H), k3, cosb, MUL
                        )
                        red = smA.tile([128, H], F32, tag="red")
                        nc.vector.tensor_reduce(
                            red[:],
                            sq[:].rearrange("p (h f) -> p h f", h=H),
                            mybir.AxisListType.X,
                            ADD,
                        )
                        lnr = smA.tile([128, H], F32, tag="lnr")
                        nc.scalar.activation(lnr[:], red[:], Ln)
                        rs = smA.tile([128, H], F32, tag="rs")
                        nc.scalar.activation(rs[:], lnr[:], Exp, scale=-0.5)
                        rsm = smA.tile([128, H], F32, tag="rsm")
                        nc.vector.tensor_scalar_mul(
                            rsm[:], rs[:], maskC_t[:, ct : ct + 1]
                        )
                        m2 = sb1.tile([128, DIM], F16, tag="m2")
                        nc.vector.tensor_tensor(
                            m2[:].rearrange("p (h g two) -> p h g two", h=H, two=2),
                            k_sw,
                            sinb4,
                            MUL,
                        )
                        s = sb1.tile([128, DIM], F16, tag="s")
                        nc.vector.tensor_tensor(s[:], m1[:], m2[:], ADD)
                        khat = sbA.tile([128, DIM], F16, tag="khat")
                        rsb = rsm[:].unsqueeze(2).broadcast_to([128, H, HD])
                        nc.vector.tensor_tensor(
                            khat[:].rearrange("p (h f) -> p h f", h=H),
                            s[:].rearrange("p (h f) -> p h f", h=H),
                            rsb,
                            MUL,
                        )

                        # kv Grams are issued one iteration late (software
                        # pipelining) so PE never waits on the khat chain
                        kv_pending.append((ct, khat, v_sb))
                        if len(kv_pending) > 1:
                            _emit_kv(kv_pending.pop(0))

                    while kv_pending:
                        _emit_kv(kv_pending.pop(0))

                    # evict kv partials and run the collective
                    kv_sb = sbA.tile([128, NPAIR * 128], F32, tag="kv_sb")
                    nc.vector.tensor_copy(kv_sb[:], kv_ps[:])
                    nc.sync.dma_start(out=kv_in_d.ap(), in_=kv_sb[:])
                    if sim_mode:
                        # stand-in for the AllReduce so TimelineSim can run
                        nc.sync.dma_start(out=kv_out_d.ap(), in_=kv_in_d.ap())
                    else:
                        nc.gpsimd.collective_compute(
                            "AllReduce",
                            ADD,
                            replica_groups=[[0, 1, 2, 3], [4, 5, 6, 7]],
                            ins=[kv_in_d.ap().opt()],
                            outs=[kv_out_d.ap().opt()],
                        )

                # kvblk: load reduced Grams, cast to fp16 block-diag
                kvblk = kvblk_pool.tile([128, NPAIR * 128], F16, tag="kvblk")
                if "C" in phases:
                    kvf = kvblk_pool.tile([128, NPAIR * 128], F32, tag="kvf")
                    nc.scalar.dma_start(out=kvf[:], in_=kv_out_d.ap())
                    nc.vector.memset(kvblk[:], 0.0)
                    # top-left diag blocks of each pair, then bottom-right
                    nc.vector.tensor_copy(
                        kvblk[0:64, :].rearrange("p (t f) -> p t f", t=NPAIR)[
                            :, :, 0:64
                        ],
                        kvf[0:64, :].rearrange("p (t f) -> p t f", t=NPAIR)[
                            :, :, 0:64
                        ],
                    )
                    nc.vector.tensor_copy(
                        kvblk[64:128, :].rearrange("p (t f) -> p t f", t=NPAIR)[
                            :, :, 64:128
                        ],
                        kvf[64:128, :].rearrange("p (t f) -> p t f", t=NPAIR)[
                            :, :, 64:128
                        ],
                    )

                # ==== Fused phase B+C: q proj/norm/rope + attn + out proj ===
                with ExitStack() as ctxB:
                  if "B" in phases and "C" in phases:
                    psB = ctxB.enter_context(
                        tc.tile_pool(name="psB", bufs=2, space="PSUM")
                    )
                    psN = ctxB.enter_context(
                        tc.tile_pool(name="psN", bufs=1, space="PSUM")
                    )
                    psAt = ctxB.enter_context(
                        tc.tile_pool(name="psAt", bufs=1, space="PSUM")
                    )
                    psO = ctxB.enter_context(
                        tc.tile_pool(name="psO", bufs=2, space="PSUM")
                    )
                    sbB = ctxB.enter_context(tc.tile_pool(name="sbB", bufs=3))
                    sbS = ctxB.enter_context(
                        tc.tile_pool(name="sbS", bufs=NJ + 1)
                    )
                    sbQ = ctxB.enter_context(tc.tile_pool(name="sbQ", bufs=2))
                    sbAt = ctxB.enter_context(
                        tc.tile_pool(name="sbAt", bufs=NJ + 2)
                    )

                    def _emit_attn_out(item):
                        ct_, qh_ = item
                        cs_ = slice(ct_ * 512, (ct_ + 1) * 512)
                        attn_sb = []
                        for hp in range(NPAIR):
                            a_ps = psAt.tile([128, 512], F32, tag="a_ps")
                            nc.tensor.matmul(
                                a_ps[:],
                                kvblk[:, hp * 128 : (hp + 1) * 128],
                                qh_[:, hp * 512 : (hp + 1) * 512],
                                start=True,
                                stop=True,
                            )
                            a_sb = sbAt.tile([128, 512], F16, tag="a_sb")
                            if hp % 2 == 0:
                                nc.scalar.activation(a_sb[:], a_ps[:], Copy)
                            else:
                                nc.vector.tensor_copy(a_sb[:], a_ps[:])
                            attn_sb.append(a_sb)

                        o_all = sbQ.tile([128, NJ * 512], F32, tag="o_all")
                        for et in range(NJ):
                            elo = et * 128
                            o_ps = psO.tile([128, 512], F32, tag="o_ps")
                            for jt in range(NJ):
                                nc.tensor.matmul(
                                    o_ps[:],
                                    wo_all[
                                        :, jt * DIM + elo : jt * DIM + elo + 128
                                    ],
                                    attn_sb[jt][:],
                                    start=(jt == 0),
                                    stop=(jt == NJ - 1),
                                )
                            nc.scalar.activation(
                                o_all[:, et * 512 : (et + 1) * 512], o_ps[:], Copy
                            )
                        nc.scalar.dma_start(
                            out=blkview(out_d, cs_),
                            in_=o_all[:].rearrange("p (t c) -> p t c", t=NJ),
                        )

                    at_pending = []
                    for ct in range(NQ_T):
                        cs = slice(ct * 512, (ct + 1) * 512)
                        norms_ps = psN.tile([16, 512], F32, tag="norms")
                        qh_all = sbQ.tile([128, NJ * 512], F16, tag="qhall")
                        q_sbs = []
                        # pass 1: projections + squares + norm accumulation
                        for jt in range(NJ):
                            jlo = jt * 128
                            q_ps = psB.tile([128, 512], F32, tag="q_ps")
                            for dc in range(ND):
                                nc.tensor.matmul(
                                    q_ps[:],
                                    wq_all[
                                        :, dc * DIM + jlo : dc * DIM + jlo + 128
                                    ],
                                    xsl(dc, cs),
                                    start=(dc == 0),
                                    stop=(dc == ND - 1),
                                )
                            q_sb = sbS.tile([128, 512], F16, tag="q_sb")
                            nc.scalar.activation(q_sb[:], q_ps[:], Copy)
                            sq = sbB.tile([128, 512], F16, tag="sqB")
                            nc.vector.tensor_mul(sq[:], q_sb[:], q_sb[:])
                            nc.tensor.matmul(
                                norms_ps[:],
                                ind16T_t[:, jt * 16 : (jt + 1) * 16],
                                sq[:],
                                start=(jt == 0),
                                stop=(jt == NJ - 1),
                            )
                            q_sbs.append(q_sb)

                        lnn = sbB.tile([16, 512], F32, tag="lnn")
                        nc.scalar.activation(lnn[:], norms_ps[:], Ln)
                        rs16 = sbB.tile([16, 512], F16, tag="rs16")
                        nc.scalar.activation(rs16[:], lnn[:], Exp, scale=-0.5)

                        # pass 2: rotation + rope + scale into qh_all
                        for jt in range(NJ):
                            q_sb = q_sbs[jt]
                            rot_ps = psB.tile([128, 512], F32, tag="rotrep")
                            nc.tensor.matmul(
                                rot_ps[:], P_t[:], q_sb[:], start=True, stop=True
                            )
                            rep_ps = psB.tile([128, 512], F32, tag="rotrep")
                            nc.tensor.matmul(
                                rep_ps[:],
                                ind16_t[:, jt * 128 : (jt + 1) * 128],
                                rs16[:],
                                start=True,
                                stop=True,
                            )
                            t1 = sbB.tile([128, 512], F16, tag="t1")
                            nc.vector.tensor_tensor(
                                t1[:], q_sb[:], cosF_t[:, cs], MUL
                            )
                            t2 = sbB.tile([128, 512], F16, tag="t2")
                            nc.vector.tensor_tensor(
                                t2[:], rot_ps[:], sinF_t[:, cs], MUL
                            )
                            s = sbB.tile([128, 512], F16, tag="sB")
                            nc.vector.tensor_tensor(s[:], t1[:], t2[:], ADD)
                            nc.vector.tensor_tensor(
                                qh_all[:, jt * 512 : (jt + 1) * 512],
                                s[:],
                                rep_ps[:],
                                MUL,
                            )

                        at_pending.append((ct, qh_all))
                        if len(at_pending) > 1:
                            _emit_attn_out(at_pending.pop(0))

                    while at_pending:
                        _emit_attn_out(at_pending.pop(0))

    nc.compile()
    return nc


_NC_CACHE = None


def _get_nc():
    global _NC_CACHE
    if _NC_CACHE is None:
        _NC_CACHE = build_nc()
    return _NC_CACHE


def make_in_maps(x, mask, Wq, Wk, Wv, Wo, norm_const):
    x = np.asarray(x, np.float32)
    mask = np.asarray(mask)
    Wq = np.asarray(Wq, np.float32)
    Wk = np.asarray(Wk, np.float32)
    Wv = np.asarray(Wv, np.float32)
    Wo = np.asarray(Wo, np.float32)
    norm_const = np.asarray(norm_const, np.float32).reshape(H)

    sig = 1.0 / (1.0 + np.exp(-norm_const.astype(np.float64)))
    svec = np.float64(C) ** (-sig)  # [H]
    s_cols = np.repeat(svec, HD)  # [DIM]

    f16 = np.float16
    WkT = np.ascontiguousarray(Wk.T).astype(f16)
    WvT = np.ascontiguousarray((Wv * s_cols[:, None].astype(np.float32)).T).astype(
        f16
    )
    WqT = np.ascontiguousarray(Wq.T).astype(f16)
    WoT = np.ascontiguousarray(Wo.T).astype(f16)

    inv_freq = 1.0 / (
        ROPE_THETA ** (np.arange(0, HD, 2, dtype=np.float64) / HD)
    )  # [32]
    freq_of_j = np.repeat(inv_freq, 2)  # [64] interleaved

    ind16T = np.zeros((DIM, 16), f16)
    for jt in range(NJ):
        for kk in range(128):
            ind16T[jt * 128 + kk, 2 * jt + (kk >= 64)] = 1.0

    ind16 = np.zeros((16, DIM), f16)
    for jt in range(NJ):
        for m in range(128):
            ind16[2 * jt + (m >= 64), jt * 128 + m] = 1.0

    Pmat = np.zeros((128, 128), f16)
    for i in range(64):
        Pmat[2 * i + 1, 2 * i] = -1.0  # out[2i] = -q[2i+1]
        Pmat[2 * i, 2 * i + 1] = 1.0  # out[2i+1] = q[2i]

    in_maps = []
    for core in range(N_CORES):
        b = core // (N_CORES // B)
        cc = core % (N_CORES // B)
        c0 = cc * R
        pos = (c0 + np.arange(R)).astype(np.float64)

        xTc = np.ascontiguousarray(x[b, c0 : c0 + R, :].T).astype(f16)

        angC = pos[:, None] * freq_of_j[None, :]  # [R, 64]
        cosCc = np.cos(angC).astype(f16)
        sinCc = np.sin(angC).astype(np.float32)
        # sign fold for the swap formulation: even j -> -sin, odd j -> +sin
        sinCc[:, 0::2] *= -1.0
        sinCc = sinCc.astype(f16)

        angF = freq_of_j[:, None] * pos[None, :]  # [64, R]
        angF2 = np.concatenate([angF, angF], axis=0)  # [128, R]
        cosFc = np.cos(angF2).astype(f16)
        sinFc = np.sin(angF2).astype(f16)

        mrow = mask[b, c0 : c0 + R].astype(np.float32)  # [R]
        maskCc = np.ascontiguousarray(mrow.reshape(NC_T, 128).T)  # [128, NC_T]

        in_maps.append(
            {
                "xT": xTc,
                "WkT": WkT,
                "WvT": WvT,
                "WqT": WqT,
                "WoT": WoT,
                "cosC": cosCc,
                "sinC": sinCc,
                "cosF": cosFc,
                "sinF": sinFc,
                "maskC": maskCc,
                "ind16T": ind16T,
                "ind16": ind16,
                "Pmat": Pmat,
            }
        )
    return in_maps


def assemble_output(results, mask):
    out = np.empty((B, C, DIM), np.float32)
    for core in range(N_CORES):
        b = core // (N_CORES // B)
        cc = core % (N_CORES // B)
        c0 = cc * R
        out[b, c0 : c0 + R, :] = results[core]["out"].T
    # q-side mask: masked rows produce zero output
    out *= np.asarray(mask)[:, :, None].astype(np.float32)
    return out


def kernel(x, mask, Wq, Wk, Wv, Wo, norm_const):
    nc = _get_nc()
    in_maps = make_in_maps(x, mask, Wq, Wk, Wv, Wo, norm_const)
    res = run_bass_kernel_spmd(nc, in_maps, list(range(N_CORES)))
    return assemble_output(res.results, mask)


# revision 27
# speedup vs baseline: 13586.9317x; 1.4231x over previous
"""Trainium2 Bass kernel for nn_Attention_43413529428606 (linear attention
with l2-normed q/k, interleaved RoPE, mask, per-head power scaling).

Sharding: the 16384 (batch*seq) rows are split across 8 NeuronCores, 2048
rows each; cores 0-3 take batch 0, cores 4-7 batch 1.  Each core computes
q/k/v projections for its rows (all 16 heads), applies l2norm+RoPE+mask,
accumulates the per-head k^T v state, AllReduces that state (512 KB) within
its batch group, then applies attention and the output projection for its
rows.  The data path is fp16 (fp32 PSUM accumulation); q/attn/out phases
are fused per 512-row supertile so nothing spills to DRAM.  The q-side
mask is applied host-side on the output rows.

Self-contained: hardcodes all shapes; no sibling imports.
"""

import sys

for _p in ("/opt/trn_rl_repo",):
    if _p not in sys.path:
        sys.path.append(_p)

from contextlib import ExitStack

import numpy as np

import concourse.bass as bass
import concourse.bacc as bacc
import concourse.tile as tile
from concourse import mybir
from concourse.bass_utils import run_bass_kernel_spmd

F32 = mybir.dt.float32
F16 = mybir.dt.float16

DIM = 1024
H = 16
HD = 64
B = 2
C = 8192
ROPE_THETA = 10000.0

N_CORES = 8
R = (B * C) // N_CORES  # 2048 rows per core
NC_T = R // 128  # 16 c-tiles of 128 (phase A)
NQ_T = R // 512  # 4 c-supertiles of 512 (fused q/attn/out phase)
ND = DIM // 128  # 8 d-chunks
NJ = DIM // 128  # 8 j-tiles
NPAIR = H // 2  # 8 head pairs

Copy = mybir.ActivationFunctionType.Copy
Square = mybir.ActivationFunctionType.Square
Ln = mybir.ActivationFunctionType.Ln
Exp = mybir.ActivationFunctionType.Exp
MUL = mybir.AluOpType.mult
ADD = mybir.AluOpType.add


def build_nc(sim_mode=False, phases="ABC", reps=1):
    nc = bacc.Bacc(
        "TRN2",
        target_bir_lowering=False,
        debug=False,
        num_devices=1 if sim_mode else N_CORES,
    )

    # ---- DRAM parameters (per-core shapes, fp16 data path) ----
    xT = nc.dram_tensor("xT", [DIM, R], F16, kind="ExternalInput").ap()
    WkT = nc.dram_tensor("WkT", [DIM, DIM], F16, kind="ExternalInput").ap()
    WvT = nc.dram_tensor("WvT", [DIM, DIM], F16, kind="ExternalInput").ap()
    WqT = nc.dram_tensor("WqT", [DIM, DIM], F16, kind="ExternalInput").ap()
    WoT = nc.dram_tensor("WoT", [DIM, DIM], F16, kind="ExternalInput").ap()
    cosC = nc.dram_tensor("cosC", [R, HD], F16, kind="ExternalInput").ap()
    sinC = nc.dram_tensor("sinC", [R, HD], F16, kind="ExternalInput").ap()
    cosF = nc.dram_tensor("cosF", [128, R], F16, kind="ExternalInput").ap()
    sinF = nc.dram_tensor("sinF", [128, R], F16, kind="ExternalInput").ap()
    maskC = nc.dram_tensor("maskC", [128, NC_T], F32, kind="ExternalInput").ap()
    ind16T = nc.dram_tensor("ind16T", [DIM, 16], F16, kind="ExternalInput").ap()
    ind16 = nc.dram_tensor("ind16", [16, DIM], F16, kind="ExternalInput").ap()
    Pmat = nc.dram_tensor("Pmat", [128, 128], F16, kind="ExternalInput").ap()

    kv_in_d = nc.dram_tensor("kv_in_d", [128, NPAIR * 128], F32)
    kv_out_d = nc.dram_tensor("kv_out_d", [128, NPAIR * 128], F32)

    out_d = nc.dram_tensor("out", [DIM, R], F32, kind="ExternalOutput").ap()

    def blkview(dram_ap, csl):
        return dram_ap.rearrange("(t p) c -> p t c", p=128)[:, :, csl]

    with tile.TileContext(nc) as tc:
        with ExitStack() as ctx:
            consts = ctx.enter_context(tc.tile_pool(name="consts", bufs=1))
            kvblk_pool = ctx.enter_context(tc.tile_pool(name="kvblk", bufs=1))

            cosC_t = consts.tile([128, NC_T * HD], F16, tag="cosC")
            sinC_t = consts.tile([128, NC_T * HD], F16, tag="sinC")
            nc.sync.dma_start(
                out=cosC_t[:].rearrange("p (t f) -> p t f", t=NC_T),
                in_=cosC[:].rearrange("(t p) f -> p t f", p=128),
            )
            nc.sync.dma_start(
                out=sinC_t[:].rearrange("p (t f) -> p t f", t=NC_T),
                in_=sinC[:].rearrange("(t p) f -> p t f", p=128),
            )
            maskC_t = consts.tile([128, NC_T], F32, tag="maskC")
            ind16T_t = consts.tile([128, NJ * 16], F16, tag="ind16T")
            ind16_t = consts.tile([16, DIM], F16, tag="ind16")
            P_t = consts.tile([128, 128], F16, tag="Pmat")
            nc.sync.dma_start(out=maskC_t[:], in_=maskC[:])
            nc.sync.dma_start(
                out=ind16T_t[:].rearrange("p (t f) -> p t f", t=NJ),
                in_=ind16T[:].rearrange("(t p) f -> p t f", p=128),
            )
            nc.sync.dma_start(out=ind16_t[:], in_=ind16[:])
            nc.sync.dma_start(out=P_t[:], in_=Pmat[:])

            for _rep in range(reps):
              with ExitStack() as ctxX:
                xpool = ctxX.enter_context(tc.tile_pool(name="xpool", bufs=1))
                xT_all = xpool.tile([128, ND * R], F16, tag="xT")
                for xc in range(4):
                    nc.sync.dma_start(
                        out=xT_all[:, xc * 2 * R : (xc + 1) * 2 * R].rearrange(
                            "p (t c) -> p t c", t=2
                        ),
                        in_=xT[xc * 256 : (xc + 1) * 256, :].rearrange(
                            "(t p) c -> p t c", p=128
                        ),
                    )

                def xsl(dc, csl):
                    lo = dc * R
                    return xT_all[:, lo + csl.start : lo + csl.stop]

                if "B" in phases and "C" in phases:
                    wBC = ctxX.enter_context(tc.tile_pool(name="wBC", bufs=1))
                    wq_all = wBC.tile([128, ND * DIM], F16, tag="wq")
                    nc.sync.dma_start(
                        out=wq_all[:].rearrange("p (t f) -> p t f", t=ND),
                        in_=WqT[:].rearrange("(t p) f -> p t f", p=128),
                    )
                    wo_all = wBC.tile([128, ND * DIM], F16, tag="wo")
                    nc.scalar.dma_start(
                        out=wo_all[:].rearrange("p (t f) -> p t f", t=ND),
                        in_=WoT[:].rearrange("(t p) f -> p t f", p=128),
                    )
                    cosF_t = wBC.tile([128, R], F16, tag="cosF")
                    sinF_t = wBC.tile([128, R], F16, tag="sinF")
                    nc.sync.dma_start(out=cosF_t[:], in_=cosF[:])
                    nc.scalar.dma_start(out=sinF_t[:], in_=sinF[:])

                # ========= Phase A: k/v proj + process + kv Grams ==========
                with ExitStack() as ctxA:
                  if "A" in phases:
                    wA = ctxA.enter_context(tc.tile_pool(name="wA", bufs=1))
                    psA = ctxA.enter_context(
                        tc.tile_pool(name="psA", bufs=3, space="PSUM")
                    )
                    pskv = ctxA.enter_context(
                        tc.tile_pool(name="pskv", bufs=1, space="PSUM")
                    )
                    sbA = ctxA.enter_context(tc.tile_pool(name="sbA", bufs=2))
                    sb1 = ctxA.enter_context(tc.tile_pool(name="sb1", bufs=2))
                    smA = ctxA.enter_context(tc.tile_pool(name="smA", bufs=2))

                    wk_all = wA.tile([128, ND * DIM], F16, tag="wk")
                    wv_all = wA.tile([128, ND * DIM], F16, tag="wv")
                    for wt, wsrc in ((wk_all, WkT), (wv_all, WvT)):
                        for xc in range(2):
                            nc.scalar.dma_start(
                                out=wt[
                                    :, xc * 4 * DIM : (xc + 1) * 4 * DIM
                                ].rearrange("p (t f) -> p t f", t=4),
                                in_=wsrc[xc * 512 : (xc + 1) * 512, :].rearrange(
                                    "(t p) f -> p t f", p=128
                                ),
                            )

                    kv_ps = pskv.tile([128, NPAIR * 128], F32, tag="kvps")
                    kv_pending = []

                    # On HW start=True zeroes the whole PSUM bank, so only
                    # the first pair written to each bank may carry it.
                    def _emit_kv(item):
                        ct_, khat_, v_ = item
                        for p in range(NPAIR):
                            ps_ = slice(p * 128, (p + 1) * 128)
                            nc.tensor.matmul(
                                kv_ps[:, ps_],
                                khat_[:, ps_],
                                v_[:, ps_],
                                start=(
                                    True
                                    if sim_mode
                                    else (ct_ == 0 and p % 4 == 0)
                                ),
                                stop=(
                                    True if sim_mode else (ct_ == NC_T - 1)
                                ),
                            )

                    for ct in range(NC_T):
                        cs = slice(ct * 128, (ct + 1) * 128)
                        k_ps = psA.tile([128, DIM], F32, tag="proj_ps")
                        v_ps = psA.tile([128, DIM], F32, tag="proj_ps")
                        for half in range(2):
                            js = slice(half * 512, (half + 1) * 512)
                            for dc in range(ND):
                                nc.tensor.matmul(
                                    k_ps[:, js],
                                    xsl(dc, cs),
                                    wk_all[
                                        :, dc * DIM + js.start : dc * DIM + js.stop
                                    ],
                                    start=(dc == 0),
                                    stop=(dc == ND - 1),
                                )
                            for dc in range(ND):
                                nc.tensor.matmul(
                                    v_ps[:, js],
                                    xsl(dc, cs),
                                    wv_all[
                                        :, dc * DIM + js.start : dc * DIM + js.stop
                                    ],
                                    start=(dc == 0),
                                    stop=(dc == ND - 1),
                                )

                        # v: evict with mask fold (per-partition scale), cast
                        v_sb = sbA.tile([128, DIM], F16, tag="v_sb")
                        nc.scalar.activation(
                            v_sb[:], v_ps[:], Copy, scale=maskC_t[:, ct : ct + 1]
                        )
                        # k: evict fast (cast fp16) to free the PSUM slot
                        k_sb = sbA.tile([128, DIM], F16, tag="k_sb")
                        nc.scalar.activation(k_sb[:], k_ps[:], Copy)
                        # squares for the l2 norm
                        sq = sbA.tile([128, DIM], F16, tag="sq")
                        nc.scalar.activation(sq[:], k_ps[:], Square)

                        cosb = (
                            cosC_t[:, ct * HD : (ct + 1) * HD]
                            .unsqueeze(1)
                            .broadcast_to([128, H, HD])
                        )
                        sinb4 = (
                            sinC_t[:, ct * HD : (ct + 1) * HD]
                            .rearrange("p (g two) -> p g two", two=2)
                            .unsqueeze(1)
                            .broadcast_to([128, H, HD // 2, 2])
                        )
                        k3 = k_sb[:].rearrange("p (h f) -> p h f", h=H)
                        k_sw = k_sb[:].rearrange(
                            "p (h g two) -> p h g two", h=H, two=2
                        )[:, :, :, ::-1]

                        m1 = sb1.tile([128, DIM], F16, tag="m1")
                        nc.vector.tensor_tensor(
                            m1[:].rearrange("p (h f) -> p h f", h=H), k3, cosb, MUL
                        )
                        red = smA.tile([128, H], F32, tag="red")
                        nc.vector.tensor_reduce(
                            red[:],
                            sq[:].rearrange("p (h f) -> p h f", h=H),
                            mybir.AxisListType.X,
                            ADD,
                        )
                        lnr = smA.tile([128, H], F32, tag="lnr")
                        nc.scalar.activation(lnr[:], red[:], Ln)
                        rs = smA.tile([128, H], F32, tag="rs")
                        nc.scalar.activation(rs[:], lnr[:], Exp, scale=-0.5)
                        rsm = smA.tile([128, H], F32, tag="rsm")
                        nc.vector.tensor_scalar_mul(
                            rsm[:], rs[:], maskC_t[:, ct : ct + 1]
                        )
                        m2 = sb1.tile([128, DIM], F16, tag="m2")
                        nc.vector.tensor_tensor(
                            m2[:].rearrange("p (h g two) -> p h g two", h=H, two=2),
                            k_sw,
                            sinb4,
                            MUL,
                        )
                        s = sb1.tile([128, DIM], F16, tag="s")
                        nc.vector.tensor_tensor(s[:], m1[:], m2[:], ADD)
                        khat = sbA.tile([128, DIM], F16, tag="khat")
                        rsb = rsm[:].unsqueeze(2).broadcast_to([128, H, HD])
                        nc.vector.tensor_tensor(
                            khat[:].rearrange("p (h f) -> p h f", h=H),
                            s[:].rearrange("p (h f) -> p h f", h=H),
                            rsb,
                            MUL,
                        )

                        # kv Grams are issued one iteration late (software
                        # pipelining) so PE never waits on the khat chain
                        kv_pending.append((ct, khat, v_sb))
                        if len(kv_pending) > 1:
                            _emit_kv(kv_pending.pop(0))

                    while kv_pending:
                        _emit_kv(kv_pending.pop(0))

                    # evict kv partials and run the collective
                    kv_sb = sbA.tile([128, NPAIR * 128], F32, tag="kv_sb")
                    nc.vector.tensor_copy(kv_sb[:], kv_ps[:])
                    nc.sync.dma_start(out=kv_in_d.ap(), in_=kv_sb[:])
                    if sim_mode:
                        # stand-in for the AllReduce so TimelineSim can run
                        nc.sync.dma_start(out=kv_out_d.ap(), in_=kv_in_d.ap())
                    else:
                        nc.gpsimd.collective_compute(
                            "AllReduce",
                            ADD,
                            replica_groups=[[0, 1, 2, 3], [4, 5, 6, 7]],
                            ins=[kv_in_d.ap().opt()],
                            outs=[kv_out_d.ap().opt()],
                        )

                # kvblk: load reduced Grams, cast to fp16 block-diag
                kvblk = kvblk_pool.tile([128, NPAIR * 128], F16, tag="kvblk")
                if "C" in phases:
                    kvf = kvblk_pool.tile([128, NPAIR * 128], F32, tag="kvf")
                    nc.scalar.dma_start(out=kvf[:], in_=kv_out_d.ap())
                    nc.vector.memset(kvblk[:], 0.0)
                    # top-left diag blocks of each pair, then bottom-right
                    nc.vector.tensor_copy(
                        kvblk[0:64, :].rearrange("p (t f) -> p t f", t=NPAIR)[
                            :, :, 0:64
                        ],
                        kvf[0:64, :].rearrange("p (t f) -> p t f", t=NPAIR)[
                            :, :, 0:64
                        ],
                    )
                    nc.vector.tensor_copy(
                        kvblk[64:128, :].rearrange("p (t f) -> p t f", t=NPAIR)[
                            :, :, 64:128
                        ],
                        kvf[64:128, :].rearrange("p (t f) -> p t f", t=NPAIR)[
                            :, :, 64:128
                        ],
                    )

                # ==== Fused phase B+C: q proj/norm/rope + attn + out proj ===
                with ExitStack() as ctxB:
                  if "B" in phases and "C" in phases:
                    psB = ctxB.enter_context(
                        tc.tile_pool(name="psB", bufs=2, space="PSUM")
                    )
                    psN = ctxB.enter_context(
                        tc.tile_pool(name="psN", bufs=1, space="PSUM")
                    )
                    psAt = ctxB.enter_context(
                        tc.tile_pool(name="psAt", bufs=1, space="PSUM")
                    )
                    psO = ctxB.enter_context(
                        tc.tile_pool(name="psO", bufs=2, space="PSUM")
                    )
                    sbB = ctxB.enter_context(tc.tile_pool(name="sbB", bufs=3))
                    sbS = ctxB.enter_context(
                        tc.tile_pool(name="sbS", bufs=2 * NJ)
                    )
                    sbQ = ctxB.enter_context(tc.tile_pool(name="sbQ", bufs=2))
                    sbAt = ctxB.enter_context(
                        tc.tile_pool(name="sbAt", bufs=NJ + 2)
                    )

                    def _emit_attn_out(item):
                        ct_, qh_ = item
                        cs_ = slice(ct_ * 512, (ct_ + 1) * 512)
                        attn_sb = []
                        for hp in range(NPAIR):
                            a_ps = psAt.tile([128, 512], F32, tag="a_ps")
                            nc.tensor.matmul(
                                a_ps[:],
                                kvblk[:, hp * 128 : (hp + 1) * 128],
                                qh_[:, hp * 512 : (hp + 1) * 512],
                                start=True,
                                stop=True,
                            )
                            a_sb = sbAt.tile([128, 512], F16, tag="a_sb")
                            if hp % 2 == 0:
                                nc.scalar.activation(a_sb[:], a_ps[:], Copy)
                            else:
                                nc.vector.tensor_copy(a_sb[:], a_ps[:])
                            attn_sb.append(a_sb)

                        o_all = sbQ.tile([128, NJ * 512], F32, tag="o_all")
                        for et in range(NJ):
                            elo = et * 128
                            o_ps = psO.tile([128, 512], F32, tag="o_ps")
                            for jt in range(NJ):
                                nc.tensor.matmul(
                                    o_ps[:],
                                    wo_all[
                                        :, jt * DIM + elo : jt * DIM + elo + 128
                                    ],
                                    attn_sb[jt][:],
                                    start=(jt == 0),
                                    stop=(jt == NJ - 1),
                                )
                            nc.scalar.activation(
                                o_all[:, et * 512 : (et + 1) * 512], o_ps[:], Copy
                            )
                        nc.scalar.dma_start(
                            out=blkview(out_d, cs_),
                            in_=o_all[:].rearrange("p (t c) -> p t c", t=NJ),
                        )

                    at_pending = []
                    for ct in range(NQ_T):
                        cs = slice(ct * 512, (ct + 1) * 512)
                        norms_ps = psN.tile([16, 512], F32, tag="norms")
                        qh_all = sbQ.tile([128, NJ * 512], F16, tag="qhall")
                        q_sbs = []
                        # pass 1: projections + squares + norm accumulation
                        for jt in range(NJ):
                            jlo = jt * 128
                            q_ps = psB.tile([128, 512], F32, tag="q_ps")
                            for dc in range(ND):
                                nc.tensor.matmul(
                                    q_ps[:],
                                    wq_all[
                                        :, dc * DIM + jlo : dc * DIM + jlo + 128
                                    ],
                                    xsl(dc, cs),
                                    start=(dc == 0),
                                    stop=(dc == ND - 1),
                                )
                            q_sb = sbS.tile([128, 512], F16, tag="q_sb")
                            nc.scalar.activation(q_sb[:], q_ps[:], Copy)
                            sq = sbB.tile([128, 512], F16, tag="sqB")
                            nc.vector.tensor_mul(sq[:], q_sb[:], q_sb[:])
                            nc.tensor.matmul(
                                norms_ps[:],
                                ind16T_t[:, jt * 16 : (jt + 1) * 16],
                                sq[:],
                                start=(jt == 0),
                                stop=(jt == NJ - 1),
                            )
                            q_sbs.append(q_sb)

                        lnn = sbB.tile([16, 512], F32, tag="lnn")
                        nc.scalar.activation(lnn[:], norms_ps[:], Ln)
                        rs16 = sbB.tile([16, 512], F16, tag="rs16")
                        nc.scalar.activation(rs16[:], lnn[:], Exp, scale=-0.5)

                        # pass 2: rotation + rope + scale into qh_all
                        for jt in range(NJ):
                            q_sb = q_sbs[jt]
                            rot_ps = psB.tile([128, 512], F32, tag="rotrep")
                            nc.tensor.matmul(
                                rot_ps[:], P_t[:], q_sb[:], start=True, stop=True
                            )
                            rep_ps = psB.tile([128, 512], F32, tag="rotrep")
                            nc.tensor.matmul(
                                rep_ps[:],
                                ind16_t[:, jt * 128 : (jt + 1) * 128],
                                rs16[:],
                                start=True,
                                stop=True,
                            )
                            t1 = sbB.tile([128, 512], F16, tag="t1")
                            nc.vector.tensor_tensor(
                                t1[:], q_sb[:], cosF_t[:, cs], MUL
                            )
                            t2 = sbB.tile([128, 512], F16, tag="t2")
                            nc.vector.tensor_tensor(
                                t2[:], rot_ps[:], sinF_t[:, cs], MUL
                            )
                            s = sbB.tile([128, 512], F16, tag="sB")
                            nc.vector.tensor_tensor(s[:], t1[:], t2[:], ADD)
                            nc.vector.tensor_tensor(
                                qh_all[:, jt * 512 : (jt + 1) * 512],
                                s[:],
                                rep_ps[:],
                                MUL,
                            )

                        at_pending.append((ct, qh_all))
                        if len(at_pending) > 1:
                            _emit_attn_out(at_pending.pop(0))

                    while at_pending:
                        _emit_attn_out(at_pending.pop(0))

    nc.compile()
    return nc


_NC_CACHE = None


def _get_nc():
    global _NC_CACHE
    if _NC_CACHE is None:
        _NC_CACHE = build_nc()
    return _NC_CACHE


def make_in_maps(x, mask, Wq, Wk, Wv, Wo, norm_const):
    x = np.asarray(x, np.float32)
    mask = np.asarray(mask)
    Wq = np.asarray(Wq, np.float32)
    Wk = np.asarray(Wk, np.float32)
    Wv = np.asarray(Wv, np.float32)
    Wo = np.asarray(Wo, np.float32)
    norm_const = np.asarray(norm_const, np.float32).reshape(H)

    sig = 1.0 / (1.0 + np.exp(-norm_const.astype(np.float64)))
    svec = np.float64(C) ** (-sig)  # [H]
    s_cols = np.repeat(svec, HD)  # [DIM]

    f16 = np.float16
    WkT = np.ascontiguousarray(Wk.T).astype(f16)
    WvT = np.ascontiguousarray((Wv * s_cols[:, None].astype(np.float32)).T).astype(
        f16
    )
    WqT = np.ascontiguousarray(Wq.T).astype(f16)
    WoT = np.ascontiguousarray(Wo.T).astype(f16)

    inv_freq = 1.0 / (
        ROPE_THETA ** (np.arange(0, HD, 2, dtype=np.float64) / HD)
    )  # [32]
    freq_of_j = np.repeat(inv_freq, 2)  # [64] interleaved

    ind16T = np.zeros((DIM, 16), f16)
    for jt in range(NJ):
        for kk in range(128):
            ind16T[jt * 128 + kk, 2 * jt + (kk >= 64)] = 1.0

    ind16 = np.zeros((16, DIM), f16)
    for jt in range(NJ):
        for m in range(128):
            ind16[2 * jt + (m >= 64), jt * 128 + m] = 1.0

    Pmat = np.zeros((128, 128), f16)
    for i in range(64):
        Pmat[2 * i + 1, 2 * i] = -1.0  # out[2i] = -q[2i+1]
        Pmat[2 * i, 2 * i + 1] = 1.0  # out[2i+1] = q[2i]

    in_maps = []
    for core in range(N_CORES):
        b = core // (N_CORES // B)
        cc = core % (N_CORES // B)
        c0 = cc * R
        pos = (c0 + np.arange(R)).astype(np.float64)

        xTc = np.ascontiguousarray(x[b, c0 : c0 + R, :].T).astype(f16)

        angC = pos[:, None] * freq_of_j[None, :]  # [R, 64]
        cosCc = np.cos(angC).astype(f16)
        sinCc = np.sin(angC).astype(np.float32)
        # sign fold for the swap formulation: even j -> -sin, odd j -> +sin
        sinCc[:, 0::2] *= -1.0
        sinCc = sinCc.astype(f16)

        angF = freq_of_j[:, None] * pos[None, :]  # [64, R]
        angF2 = np.concatenate([angF, angF], axis=0)  # [128, R]
        cosFc = np.cos(angF2).astype(f16)
        sinFc = np.sin(angF2).astype(f16)

        mrow = mask[b, c0 : c0 + R].astype(np.float32)  # [R]
        maskCc = np.ascontiguousarray(mrow.reshape(NC_T, 128).T)  # [128, NC_T]

        in_maps.append(
            {
                "xT": xTc,
                "WkT": WkT,
                "WvT": WvT,
                "WqT": WqT,
                "WoT": WoT,
                "cosC": cosCc,
                "sinC": sinCc,
                "cosF": cosFc,
                "sinF": sinFc,
                "maskC": maskCc,
                "ind16T": ind16T,
                "ind16": ind16,
                "Pmat": Pmat,
            }
        )
    return in_maps


def assemble_output(results, mask):
    out = np.empty((B, C, DIM), np.float32)
    for core in range(N_CORES):
        b = core // (N_CORES // B)
        cc = core % (N_CORES // B)
        c0 = cc * R
        out[b, c0 : c0 + R, :] = results[core]["out"].T
    # q-side mask: masked rows produce zero output
    out *= np.asarray(mask)[:, :, None].astype(np.float32)
    return out


def kernel(x, mask, Wq, Wk, Wv, Wo, norm_const):
    nc = _get_nc()
    in_maps = make_in_maps(x, mask, Wq, Wk, Wv, Wo, norm_const)
    res = run_bass_kernel_spmd(nc, in_maps, list(range(N_CORES)))
    return assemble_output(res.results, mask)
